# revision 4
# baseline (speedup 1.0000x reference)
"""Trainium2 Bass kernel for nn_Attention_9457517985916.

Multi-stage attention (Global/Context/Composition/Output) + GLU fusion.
B=128, N=196, RNN=2048, ATT=1024 on 8 NeuronCores, data-parallel over B
(16 rows per core, weights replicated). All activations/weights are laid
out on the host so every device matmul contracts along the partition
axis with unit-stride DMA loads:

  - roiT   [RNN, 3136]   d-major rows for the big matmul R^T = wvp^T @ roiT
  - roinat [3136, RNN]   n-major rows for the attention-weighted row sums
  - proiT  [16, ATT, N]  att-major for the global score pass
  - score passes run as ACT tanh(in*scale + bias_col) followed by
    float32r matmuls against the wa vectors (contraction over ATT on
    the partition axis)
  - R^T (25088 x 1024 worth of projected roi features) bounces through
    DRAM as bf16 to stay inside the SBUF budget
  - GLU is computed in two halves: the h-dependent half early (overlaps
    the big matmul), the output-dependent half at the tail.

No collectives are used; every graded output is a per-core row slice
that the host reassembles.
"""

from contextlib import ExitStack

import numpy as np
import ml_dtypes

import concourse.bass as bass
import concourse.tile as tile
from concourse import bacc, mybir
from concourse.bass_utils import run_bass_kernel_spmd
from concourse.masks import make_identity

F32 = mybir.dt.float32
BF16 = mybir.dt.bfloat16
F32R = mybir.dt.float32r
AF = mybir.ActivationFunctionType
ALU = mybir.AluOpType

B, N, RNN, ATT = 128, 196, 2048, 1024
NCORES = 8
BL = B // NCORES          # 16 rows per core
ROWS = BL * N             # 3136
KT = RNN // 128           # 16 d-chunks
ACH = ATT // 128          # 8 att-chunks
NPAIR = BL // 2           # 8 row pairs
GRP = 2 * N               # 392 big-matmul rows per group (2 batch rows)
NGRP = ROWS // GRP        # 8

# dtype knobs (bf16 streams keep HBM traffic at ~compute parity)
BIG = BF16     # roiT / wvp / big matmul + cvp
EIN = BF16     # roinat / einsum matmuls
PROI = BF16    # p_roi stream
SML = BF16     # wh4 / wv2 / hT projections
GLUDT = BF16   # glu weights + glu matmuls

_NP = {F32: np.float32, F32R: np.float32, BF16: ml_dtypes.bfloat16}


def _cast(a, dt):
    return np.ascontiguousarray(np.asarray(a), dtype=_NP[dt])


def build():
    nc = bacc.Bacc("TRN2", target_bir_lowering=False, debug=False)
    dti = lambda n, s, d: nc.dram_tensor(n, s, d, kind="ExternalInput").ap()
    dto = lambda n, s, d: nc.dram_tensor(n, s, d, kind="ExternalOutput").ap()

    roiT = dti("roiT", [RNN, ROWS], BIG)
    roinat = dti("roinat", [ROWS, RNN], EIN)
    proiT = dti("proiT", [BL, ATT, N], PROI)
    hT = dti("hT", [RNN, BL], SML)
    ctxT = dti("ctxT", [RNN, BL], F32)
    wvp = dti("wvp", [RNN, ATT], BIG)
    wh4 = dti("wh4", [4, RNN, ATT], SML)
    wv2 = dti("wv2", [2, RNN, ATT], SML)
    wa4 = dti("wa4", [4, ATT], F32R)
    wb4 = dti("wb4", [4, ATT], F32)
    wvb3 = dti("wvb3", [3, ATT], F32)
    gluw = dti("gluw", [2 * RNN, 2 * RNN], GLUDT)
    glub = dti("glub", [1, 2 * RNN], GLUDT)
    mp = dti("mp", [1, 2 * BL], F32)

    xs_o = dto("xs", [BL, RNN], F32)       # [16, 2048] x rows
    on_o = dto("onat", [BL, RNN], F32)     # [16, 2048] output rows
    gw_o = dto("gw", [BL, N], F32)
    cw_o = dto("cw", [BL, 2], F32)
    pw_o = dto("pw", [BL, N], F32)
    ow_o = dto("ow", [BL, 2], F32)

    with tile.TileContext(nc) as tc, ExitStack() as ctx:
        const = ctx.enter_context(tc.tile_pool(name="const", bufs=1))
        wstr = ctx.enter_context(tc.tile_pool(name="wstr", bufs=2))
        rstr = ctx.enter_context(tc.tile_pool(name="rstr", bufs=2))
        pstr = ctx.enter_context(tc.tile_pool(name="pstr", bufs=3))
        work = ctx.enter_context(tc.tile_pool(name="work", bufs=2))
        ttp = ctx.enter_context(tc.tile_pool(name="ttp", bufs=4))
        pmm = ctx.enter_context(tc.tile_pool(name="pmm", bufs=2, space="PSUM"))
        pvec = ctx.enter_context(tc.tile_pool(name="pvec", bufs=3, space="PSUM"))
        ptr = ctx.enter_context(tc.tile_pool(name="ptr", bufs=2, space="PSUM"))
        dram = ctx.enter_context(tc.tile_pool(name="dram", bufs=1, space="DRAM"))

        # ---- constants ----
        ident = const.tile([128, 128], F32, tag="ident")
        make_identity(nc, ident)
        ones_g = const.tile([1, BL], GLUDT, tag="ones_g")
        nc.vector.memset(ones_g, 1.0)
        ones_c = const.tile([1, 128], F32, tag="ones_c")
        nc.vector.memset(ones_c, 1.0)

        wa_sb = const.tile([128, 4, ACH], F32R, tag="wa_sb")
        nc.sync.dma_start(wa_sb, wa4.rearrange("j (ch p) -> p j ch", p=128))
        wb_sb = const.tile([128, 4, ACH], F32, tag="wb_sb")
        nc.sync.dma_start(wb_sb, wb4.rearrange("j (ch p) -> p j ch", p=128))
        wvb_sb = const.tile([128, 3, ACH], F32, tag="wvb_sb")
        nc.sync.dma_start(wvb_sb, wvb3.rearrange("j (ch p) -> p j ch", p=128))
        mp_sb = const.tile([1, 2 * BL], F32, tag="mp_sb")
        nc.sync.dma_start(mp_sb, mp)
        glub_sb = const.tile([1, 2 * RNN], GLUDT, tag="glub_sb")
        nc.sync.dma_start(glub_sb, glub)

        hT_sb = const.tile([128, KT, BL], SML, tag="hT_sb")
        nc.sync.dma_start(hT_sb, hT.rearrange("(kt p) b -> p kt b", p=128))
        ctxT_sb = const.tile([128, KT, BL], F32, tag="ctxT_sb")
        nc.sync.dma_start(ctxT_sb, ctxT.rearrange("(kt p) b -> p kt b", p=128))
        wvp_sb = const.tile([128, KT, ATT], BIG, tag="wvp_sb")
        nc.sync.dma_start(wvp_sb, wvp.rearrange("(kt p) m -> p kt m", p=128))

        # persistent state
        hT4 = const.tile([128, 4, ACH, BL], F32, tag="hT4")
        globT = const.tile([128, KT, BL], F32, tag="globT")
        ctxoT = const.tile([128, KT, BL], F32, tag="ctxoT")
        hpc = const.tile([128, ACH, BL], F32, tag="hpc")
        compT = const.tile([128, KT, BL], F32, tag="compT")
        outT = const.tile([128, KT, BL], F32, tag="outT")

        RT_d = dram.tile([ACH, NGRP, 128, GRP], BF16, tag="RT_d")
        zh_d = dram.tile([BL, 2 * RNN], F32, tag="zh_d")

        gwT0, gwT1, pwT0, pwT1 = {}, {}, {}, {}
        for b in range(BL):
            for d, nm in ((gwT0, "gwT0"), (gwT1, "gwT1"), (pwT0, "pwT0"), (pwT1, "pwT1")):
                d[b] = const.tile(
                    [128, 1], EIN, tag=f"{nm}_{b}", name=f"{nm}_{b}"
                )

        # fused psum->sbuf transpose helper: out_ap = in_sb^T (+bias +extra)
        def transpose_cb(in_sb, out_ap, bias=None, extra_add=None):
            p_in = in_sb.shape[0]
            f_in = in_sb.shape[1]
            tr = ptr.tile([128, 128], F32, tag="ptr")
            trv = tr[:f_in, :p_in]
            nc.tensor.transpose(trv, in_sb, ident[:p_in, :p_in])
            if extra_add is not None:
                if bias is not None:
                    tmp = ttp.tile([128, BL], F32, tag="trtmp")
                    tv = tmp[:f_in, :p_in]
                    nc.scalar.add(tv, trv, bias[:f_in, :])
                    nc.vector.tensor_add(out_ap, tv, extra_add)
                else:
                    nc.vector.tensor_add(out_ap, trv, extra_add)
            elif bias is not None:
                nc.scalar.add(out_ap, trv, bias[:f_in, :])
            else:
                nc.scalar.copy(out_ap, trv)

        # ---- S1: h projections (hg,hc,hp,ho) -> hT4[:, j, ch, :] ----
        for j in range(4):
            hn = work.tile([2 * BL, ATT], F32, tag="projnat")
            for ng in range(2):
                ps = pmm.tile([BL, 512], F32, tag="pmm")
                for kh in range(2):
                    wt = wstr.tile([128, KT // 2, 512], SML, tag="wst")
                    nc.sync.dma_start(
                        wt,
                        wh4[j].rearrange("(kt p) m -> p kt m", p=128)[
                            :, kh * 8 : (kh + 1) * 8, ng * 512 : (ng + 1) * 512
                        ],
                    )
                    for k8 in range(KT // 2):
                        kt = kh * 8 + k8
                        nc.tensor.matmul(
                            ps, hT_sb[:, kt, :], wt[:, k8, :],
                            start=(kt == 0), stop=(kt == KT - 1),
                        )
                nc.vector.tensor_copy(hn[:BL, ng * 512 : (ng + 1) * 512], ps)
            for ch in range(ACH):
                transpose_cb(
                    hn[:BL, ch * 128 : (ch + 1) * 128],
                    hT4[:, j, ch, :],
                    bias=wb_sb[:, j, ch : ch + 1],
                )

        # softmax over one [1, N] psum slice -> f32 row out + bf16 transposed
        # column chunks for the einsum stationaries
        def softmax_row(ps_half, out_dram_row, t0, t1):
            nm = ttp.tile([1, 1], F32, tag="nm")
            nc.vector.tensor_reduce(
                out=nm, in_=ps_half, op=ALU.max, axis=mybir.AxisListType.X, negate=True
            )
            ex = ttp.tile([1, N], F32, tag="ex")
            se = ttp.tile([1, 1], F32, tag="se")
            nc.scalar.activation(ex, ps_half, AF.Exp, bias=nm, scale=1.0, accum_out=se)
            rc = ttp.tile([1, 1], F32, tag="rc")
            nc.vector.reciprocal(rc, se)
            wf = ttp.tile([1, N], F32, tag="wf")
            nc.vector.tensor_scalar_mul(wf, ex, rc)
            nc.gpsimd.dma_start(out_dram_row, wf)
            tra = ptr.tile([128, 128], F32, tag="ptr")
            nc.tensor.transpose(tra[:128, :1], wf[:, :128], ident[:1, :1])
            nc.vector.tensor_copy(t0, tra[:128, :1])
            trb = ptr.tile([128, 128], F32, tag="ptr")
            nc.tensor.transpose(trb[: N - 128, :1], wf[:, 128:N], ident[:1, :1])
            nc.vector.tensor_copy(t1[: N - 128, :], trb[: N - 128, :1])

        # ---- S2: global attention scores -> gw ----
        for pb in range(NPAIR):
            ps_s = pvec.tile([1, GRP], F32, tag="pv")
            for ch in range(ACH):
                tt = ttp.tile([128, GRP], F32R, tag="tt")
                for h in range(2):
                    b = 2 * pb + h
                    pt = pstr.tile([128, N], PROI, tag="pt")
                    nc.sync.dma_start(pt, proiT[b, ch * 128 : (ch + 1) * 128, :])
                    nc.scalar.activation(
                        tt[:, h * N : (h + 1) * N], pt, AF.Tanh,
                        bias=hT4[:, 0, ch, b : b + 1], scale=1.0,
                    )
                nc.tensor.matmul(
                    ps_s, wa_sb[:, 0, ch : ch + 1], tt,
                    start=(ch == 0), stop=(ch == ACH - 1),
                )
            for h in range(2):
                b = 2 * pb + h
                softmax_row(
                    ps_s[:, h * N : (h + 1) * N], gw_o[b : b + 1, :], gwT0[b], gwT1[b]
                )

        # ---- S3: big matmul R^T = wvp^T @ roiT -> DRAM (bf16) ----
        roiT_r = roiT.rearrange("(kt p) r -> p kt r", p=128)
        for g in range(NGRP):
            rt = rstr.tile([128, KT, GRP], BIG, tag="rt")
            nc.sync.dma_start(rt, roiT_r[:, :, g * GRP : (g + 1) * GRP])
            for m in range(ACH):
                ps = pmm.tile([128, GRP], F32, tag="pmm")
                for kt in range(KT):
                    nc.tensor.matmul(
                        ps, wvp_sb[:, kt, m * 128 : (m + 1) * 128], rt[:, kt, :],
                        start=(kt == 0), stop=(kt == KT - 1),
                    )
                rc_sb = work.tile([128, GRP], BF16, tag="rtcb")
                nc.vector.tensor_copy(rc_sb, ps)
                nc.sync.dma_start(RT_d[m, g], rc_sb)

        # ---- S10a: GLU h-half (z_h = h @ glu_w[RNN:, :]) -> DRAM spill ----
        gluw_r = gluw.rearrange("(kt p) c -> p kt c", p=128)
        for ng in range(8):
            ps = pmm.tile([BL, 512], F32, tag="pmm")
            for kh in range(2):
                gt = wstr.tile([128, KT // 2, 512], GLUDT, tag="wst")
                nc.sync.dma_start(
                    gt,
                    gluw_r[:, KT + kh * 8 : KT + (kh + 1) * 8,
                           ng * 512 : (ng + 1) * 512],
                )
                for k8 in range(KT // 2):
                    kt = kh * 8 + k8
                    nc.tensor.matmul(
                        ps, hT_sb[:, kt, :], gt[:, k8, :],
                        start=(kt == 0), stop=(kt == KT - 1),
                    )
            zt = work.tile([BL, 512], F32, tag="zt")
            nc.vector.tensor_copy(zt, ps)
            nc.sync.dma_start(zh_d[:, ng * 512 : (ng + 1) * 512], zt)

        # ---- einsum: out_nat[b, :] = sum_n w[b, n] * roi[b, n, :] ----
        def einsum_rows(wT0, wT1, nat_out):
            for b in range(BL):
                for dg in range(4):
                    ra = rstr.tile([128, 512], EIN, tag="ra")
                    nc.sync.dma_start(
                        ra, roinat[b * N : b * N + 128, dg * 512 : (dg + 1) * 512]
                    )
                    rb = rstr.tile([128, 512], EIN, tag="rb")
                    nc.sync.dma_start(
                        rb[: N - 128, :],
                        roinat[b * N + 128 : (b + 1) * N, dg * 512 : (dg + 1) * 512],
                    )
                    pe = pvec.tile([1, 512], F32, tag="pv")
                    nc.tensor.matmul(pe, wT0[b], ra, start=True, stop=False)
                    nc.tensor.matmul(
                        pe, wT1[b][: N - 128, :], rb[: N - 128, :],
                        start=False, stop=True,
                    )
                    grow = work.tile([1, 512], F32, tag="grow")
                    nc.vector.tensor_copy(grow, pe)
                    nc.sync.dma_start(
                        nat_out[b : b + 1, dg * 512 : (dg + 1) * 512], grow
                    )

        # ---- S4: global_out ----
        gnat = work.tile([BL, RNN], F32, tag="natrow")
        einsum_rows(gwT0, gwT1, gnat)
        for kt in range(KT):
            transpose_cb(gnat[:, kt * 128 : (kt + 1) * 128], globT[:, kt, :])

        # ---- S5: context attention ----
        r2 = const.tile([128, KT, 2 * BL], SML, tag="r2")
        for kt in range(KT):
            r2v = r2[:, kt, :].rearrange("p (b k) -> p b k", k=2)
            nc.vector.tensor_copy(r2v[:, :, 0], globT[:, kt, :])
            nc.vector.tensor_copy(r2v[:, :, 1], ctxT_sb[:, kt, :])

        def kv2_attention(stat_sb, wj, vbj, hj, score_j, mask_sb, w_out_dram):
            """2-key attention over stat_sb [128, KT, 32] (d x (b,k))."""
            vnat = work.tile([2 * BL, ATT], F32, tag="projnat")
            for ng in range(2):
                ps = pmm.tile([2 * BL, 512], F32, tag="pmm")
                for kh in range(2):
                    wt = wstr.tile([128, KT // 2, 512], SML, tag="wst")
                    nc.sync.dma_start(
                        wt,
                        wv2[wj].rearrange("(kt p) m -> p kt m", p=128)[
                            :, kh * 8 : (kh + 1) * 8, ng * 512 : (ng + 1) * 512
                        ],
                    )
                    for k8 in range(KT // 2):
                        kt = kh * 8 + k8
                        nc.tensor.matmul(
                            ps, stat_sb[:, kt, :], wt[:, k8, :],
                            start=(kt == 0), stop=(kt == KT - 1),
                        )
                nc.vector.tensor_copy(vnat[:, ng * 512 : (ng + 1) * 512], ps)
            ps_c = pvec.tile([1, 2 * BL], F32, tag="pv")
            for ch in range(ACH):
                vT = ttp.tile([128, 2 * BL], F32, tag="vT")
                transpose_cb(
                    vnat[:, ch * 128 : (ch + 1) * 128], vT,
                    bias=wvb_sb[:, vbj, ch : ch + 1],
                )
                ta = ttp.tile([128, 2 * BL], F32, tag="ta")
                nc.vector.tensor_add(
                    ta.rearrange("p (b k) -> p b k", k=2),
                    vT.rearrange("p (b k) -> p b k", k=2),
                    hT4[:, hj, ch, :].unsqueeze(2).broadcast_to([128, BL, 2]),
                )
                tb = ttp.tile([128, 2 * BL], F32R, tag="tb")
                nc.scalar.activation(tb, ta, AF.Tanh)
                nc.tensor.matmul(
                    ps_c, wa_sb[:, score_j, ch : ch + 1], tb,
                    start=(ch == 0), stop=(ch == ACH - 1),
                )
            ew = ttp.tile([1, 2 * BL], F32, tag="ew")
            nc.scalar.activation(ew, ps_c, AF.Exp)
            if mask_sb is not None:
                mw = ttp.tile([1, 2 * BL], F32, tag="mw")
                nc.vector.tensor_mul(mw, ew, mask_sb)
                ew = mw
            ssum = ttp.tile([1, BL], F32, tag="ssum")
            nc.vector.tensor_reduce(
                out=ssum.unsqueeze(2),
                in_=ew.rearrange("p (b k) -> p b k", k=2),
                op=ALU.add, axis=mybir.AxisListType.X,
            )
            rcp = ttp.tile([1, BL], F32, tag="rc16")
            nc.vector.reciprocal(rcp, ssum)
            wgt = ttp.tile([1, 2 * BL], F32, tag="wgt")
            nc.vector.tensor_mul(
                wgt.rearrange("p (b k) -> p b k", k=2),
                ew.rearrange("p (b k) -> p b k", k=2),
                rcp.unsqueeze(2).broadcast_to([1, BL, 2]),
            )
            nc.gpsimd.dma_start(
                w_out_dram.rearrange("b k -> (b k)").unsqueeze(0), wgt
            )
            # replicate to all partitions via ones outer product
            psr = ptr.tile([128, 128], F32, tag="ptr")
            nc.tensor.matmul(
                psr[:, : 2 * BL], ones_c[:, :128], wgt, start=True, stop=True,
            )
            wr = work.tile([128, 2 * BL], F32, tag="wrep")
            nc.vector.tensor_copy(wr, psr[:, : 2 * BL])
            return wr

        cwr = kv2_attention(r2, 0, 0, 1, 1, mp_sb, cw_o)
        cwv = cwr.rearrange("p (b k) -> p b k", k=2)
        for kt in range(KT):
            t0 = ttp.tile([128, BL], F32, tag="cmb0")
            nc.vector.tensor_mul(t0, globT[:, kt, :], cwv[:, :, 0])
            t1 = ttp.tile([128, BL], F32, tag="cmb1")
            nc.vector.tensor_mul(t1, ctxT_sb[:, kt, :], cwv[:, :, 1])
            nc.vector.tensor_add(ctxoT[:, kt, :], t0, t1)

        # ---- S6: hpc = hp + (context_out @ wvp + wvp_b)^T ----
        cxb = const.tile([128, KT, BL], BIG, tag="cxb")
        for kt in range(KT):
            nc.vector.tensor_copy(cxb[:, kt, :], ctxoT[:, kt, :])
        cvpn = work.tile([2 * BL, ATT], F32, tag="projnat")
        for ng in range(2):
            ps = pmm.tile([BL, 512], F32, tag="pmm")
            for kt in range(KT):
                nc.tensor.matmul(
                    ps, cxb[:, kt, :], wvp_sb[:, kt, ng * 512 : (ng + 1) * 512],
                    start=(kt == 0), stop=(kt == KT - 1),
                )
            nc.vector.tensor_copy(cvpn[:BL, ng * 512 : (ng + 1) * 512], ps)
        for ch in range(ACH):
            transpose_cb(
                cvpn[:BL, ch * 128 : (ch + 1) * 128], hpc[:, ch, :],
                bias=wvb_sb[:, 1, ch : ch + 1], extra_add=hT4[:, 2, ch, :],
            )

        # ---- S7: composition scores -> pw ----
        for pb in range(NPAIR):
            ps_s = pvec.tile([1, GRP], F32, tag="pv")
            for ch in range(ACH):
                tt = ttp.tile([128, GRP], F32R, tag="tt")
                for h in range(2):
                    b = 2 * pb + h
                    rtt = pstr.tile([128, N], BF16, tag="rtt")
                    nc.sync.dma_start(rtt, RT_d[ch, pb, :, h * N : (h + 1) * N])
                    nc.scalar.activation(
                        tt[:, h * N : (h + 1) * N], rtt, AF.Tanh,
                        bias=hpc[:, ch, b : b + 1], scale=-1.0,
                    )
                nc.tensor.matmul(
                    ps_s, wa_sb[:, 2, ch : ch + 1], tt,
                    start=(ch == 0), stop=(ch == ACH - 1),
                )
            for h in range(2):
                b = 2 * pb + h
                softmax_row(
                    ps_s[:, h * N : (h + 1) * N], pw_o[b : b + 1, :], pwT0[b], pwT1[b]
                )

        # ---- S8: comp_out^T = ctxo^T - sum_n pw*roi ----
        cnat = work.tile([BL, RNN], F32, tag="natrow")
        einsum_rows(pwT0, pwT1, cnat)
        for kt in range(KT):
            tr = ptr.tile([128, 128], F32, tag="ptr")
            nc.tensor.transpose(
                tr[:, :BL], cnat[:, kt * 128 : (kt + 1) * 128], ident[:BL, :BL]
            )
            nc.vector.tensor_sub(compT[:, kt, :], ctxoT[:, kt, :], tr[:, :BL])

        # ---- S9: output attention ----
        of2 = const.tile([128, KT, 2 * BL], SML, tag="of2")
        for kt in range(KT):
            ofv = of2[:, kt, :].rearrange("p (b k) -> p b k", k=2)
            nc.vector.tensor_copy(ofv[:, :, 0], globT[:, kt, :])
            nc.vector.tensor_copy(ofv[:, :, 1], compT[:, kt, :])
        owr = kv2_attention(of2, 1, 2, 3, 3, None, ow_o)
        owv = owr.rearrange("p (b k) -> p b k", k=2)
        for kt in range(KT):
            t0 = ttp.tile([128, BL], F32, tag="cmb0")
            nc.vector.tensor_mul(t0, globT[:, kt, :], owv[:, :, 0])
            t1 = ttp.tile([128, BL], F32, tag="cmb1")
            nc.vector.tensor_mul(t1, compT[:, kt, :], owv[:, :, 1])
            nc.vector.tensor_add(outT[:, kt, :], t0, t1)

        # output rows (natural layout) to DRAM
        onat = work.tile([BL, RNN], F32, tag="onat")
        for kt in range(KT):
            tr = ptr.tile([128, 128], F32, tag="ptr")
            nc.tensor.transpose(tr[:BL, :], outT[:, kt, :], ident[:128, :128])
            nc.scalar.copy(onat[:, kt * 128 : (kt + 1) * 128], tr[:BL, :])
        nc.sync.dma_start(on_o, onat)

        # ---- S10b: GLU output-half + gate ----
        outTg = const.tile([128, KT, BL], GLUDT, tag="outTg")
        for kt in range(KT):
            nc.vector.tensor_copy(outTg[:, kt, :], outT[:, kt, :])
        for ngx in range(4):
            za = work.tile([BL, 512], F32, tag="za")
            zb = work.tile([BL, 512], F32, tag="zb")
            for half, zdst in ((0, za), (1, zb)):
                ng = ngx + 4 * half
                ps = pmm.tile([BL, 512], F32, tag="pmm")
                for kh in range(2):
                    gt = wstr.tile([128, KT // 2, 512], GLUDT, tag="wst")
                    nc.sync.dma_start(
                        gt,
                        gluw_r[:, kh * 8 : (kh + 1) * 8, ng * 512 : (ng + 1) * 512],
                    )
                    for k8 in range(KT // 2):
                        kt = kh * 8 + k8
                        nc.tensor.matmul(
                            ps, outTg[:, kt, :], gt[:, k8, :],
                            start=(kt == 0), stop=False,
                        )
                nc.tensor.matmul(
                    ps, ones_g, glub_sb[:, ng * 512 : (ng + 1) * 512],
                    start=False, stop=True,
                )
                zhp = work.tile([BL, 512], F32, tag="zhp")
                nc.sync.dma_start(zhp, zh_d[:, ng * 512 : (ng + 1) * 512])
                nc.vector.tensor_add(zdst, ps, zhp)
            sg = work.tile([BL, 512], F32, tag="sg")
            nc.scalar.activation(sg, zb, AF.Sigmoid)
            xt = work.tile([BL, 512], F32, tag="xt")
            nc.vector.tensor_mul(xt, za, sg)
            nc.sync.dma_start(xs_o[:, ngx * 512 : (ngx + 1) * 512], xt)

    nc.compile()
    return nc


_NC_CACHE = None


def _get_nc():
    global _NC_CACHE
    if _NC_CACHE is None:
        _NC_CACHE = build()
    return _NC_CACHE


def prep_inputs(
    h, roi_feats, p_roi_feats, mask, context,
    whg_w, whg_b, wag_w, wag_b,
    whc_w, whc_b, wvc_w, wvc_b, wac_w, wac_b,
    whp_w, whp_b, wvp_w, wvp_b, wap_w, wap_b,
    wvo_w, wvo_b, who_w, who_b, wao_w, wao_b,
    glu_w, glu_b,
):
    """Build the 8 per-core input maps (host-side sharding + layout)."""
    h = np.asarray(h, np.float32)
    roi = np.asarray(roi_feats, np.float32)
    proi = np.asarray(p_roi_feats, np.float32)
    mask = np.asarray(mask)
    context = np.asarray(context, np.float32)

    wh4 = _cast(np.stack([whg_w, whc_w, whp_w, who_w]), SML)
    wv2 = _cast(np.stack([wvc_w, wvo_w]), SML)
    wa4 = _cast(np.stack([wag_w, wac_w, wap_w, wao_w]), F32R)
    wb4 = _cast(np.stack([whg_b, whc_b, whp_b, who_b]), F32)
    wvb3 = _cast(np.stack([wvc_b, wvp_b, wvo_b]), F32)
    wvp = _cast(wvp_w, BIG)
    gluw = _cast(glu_w, GLUDT)
    glub = _cast(np.asarray(glu_b, np.float32).reshape(1, -1), GLUDT)

    # NOTE: wag_b/wac_b/wap_b/wao_b shift scores by a constant, which cancels
    # in the softmax (also under the mask-renormalization), so they are unused.

    in_maps = []
    for c in range(NCORES):
        rows = slice(c * BL, (c + 1) * BL)
        roi_c = roi[rows].reshape(ROWS, RNN)
        cm = (mask[rows] > 0).astype(np.float32)
        mpv = np.stack([np.ones(BL, np.float32), cm], axis=1).reshape(1, 2 * BL)
        in_maps.append(
            dict(
                roiT=_cast(roi_c.T, BIG),
                roinat=_cast(roi_c, EIN),
                proiT=_cast(proi[rows].transpose(0, 2, 1), PROI),
                hT=_cast(h[rows].T, SML),
                ctxT=_cast(context[rows].T, F32),
                wvp=wvp, wh4=wh4, wv2=wv2, wa4=wa4, wb4=wb4, wvb3=wvb3,
                gluw=gluw, glub=glub, mp=mpv,
            )
        )
    return in_maps


def run_on_device(in_maps, trace=False):
    nc = _get_nc()
    return run_bass_kernel_spmd(nc, in_maps, list(range(NCORES)), trace=trace)


def assemble(results):
    x = np.concatenate([r["xs"] for r in results], axis=0).astype(np.float32)
    output = np.concatenate([r["onat"] for r in results], axis=0).astype(np.float32)
    gw = np.concatenate([r["gw"] for r in results], axis=0).astype(np.float32)
    cw = np.concatenate([r["cw"] for r in results], axis=0).astype(np.float32)
    pw = np.concatenate([r["pw"] for r in results], axis=0).astype(np.float32)
    ow = np.concatenate([r["ow"] for r in results], axis=0).astype(np.float32)
    return (x, output, gw, cw, pw, ow)


def kernel(**inputs):
    in_maps = prep_inputs(**inputs)
    res = run_on_device(in_maps, trace=False)
    return assemble(res.results)


# revision 8
# speedup vs baseline: 1.2118x; 1.2118x over previous
"""Trainium2 Bass kernel for nn_Attention_9457517985916.

Multi-stage attention (Global/Context/Composition/Output) + GLU fusion.
B=128, N=196, RNN=2048, ATT=1024 on 8 NeuronCores, data-parallel over B
(16 rows per core, weights replicated).

Layout strategy: every bulk stream is pre-tiled on the host so each DMA
lands in SBUF with maximal-contiguity descriptors (KB-scale per-partition
runs instead of the 392..784B runs a strided read would produce):

  - roiTt  [8, 128, 16, 392]      per-group d-major roi rows (big matmul)
  - roinat [3136, 2048]           n-major rows (attention-weighted sums)
  - proiT2 [16, 128, 8, 196]      att-major p_roi (global score pass)
  - wh4t/wv2t/gluwt               weight chunks pre-arranged per DMA tile

Compute structure per core:
  R^T = wvp^T @ roiT (bf16, fp32 accumulate) spills per-group to DRAM;
  score passes are ACT tanh(in*scale + bias_col) followed by float32r
  matmuls against the wa vectors; softmaxes are batched to avoid ACT
  table reloads; the attention-weighted row sums run as (196->2)-chunk
  PE matmuls over natural-layout roi; GLU runs in two halves (h-half
  early, output-half at the tail). No collectives; every graded output
  is a per-core row slice the host reassembles.
"""

from contextlib import ExitStack

import numpy as np
import ml_dtypes

import concourse.bass as bass
import concourse.tile as tile
from concourse import bacc, mybir
from concourse.bass_utils import run_bass_kernel_spmd
from concourse.masks import make_identity

F32 = mybir.dt.float32
BF16 = mybir.dt.bfloat16
F32R = mybir.dt.float32r
AF = mybir.ActivationFunctionType
ALU = mybir.AluOpType
AXX = mybir.AxisListType.X

B, N, RNN, ATT = 128, 196, 2048, 1024
NCORES = 8
BL = B // NCORES          # 16 rows per core
ROWS = BL * N             # 3136
KT = RNN // 128           # 16 d-chunks
ACH = ATT // 128          # 8 att-chunks
NPAIR = BL // 2           # 8 row pairs
GRP = 2 * N               # 392 big-matmul rows per group (2 batch rows)
NGRP = ROWS // GRP        # 8

# dtype knobs
BIG = BF16     # roiT / wvp / big matmul + cvp
EIN = BF16     # roinat / einsum matmuls
PROI = BF16    # p_roi stream
SML = BF16     # wh4 / wv2 / hT projections
GLUDT = BF16   # glu weights + glu matmuls

_NP = {F32: np.float32, F32R: np.float32, BF16: ml_dtypes.bfloat16}


def _cast(a, dt):
    return np.ascontiguousarray(np.asarray(a), dtype=_NP[dt])


def build():
    nc = bacc.Bacc("TRN2", target_bir_lowering=False, debug=False)
    dti = lambda n, s, d: nc.dram_tensor(n, s, d, kind="ExternalInput").ap()
    dto = lambda n, s, d: nc.dram_tensor(n, s, d, kind="ExternalOutput").ap()

    roiTt = dti("roiTt", [NGRP, 128, KT, GRP], BIG)
    roinat = dti("roinat", [ROWS, RNN], EIN)
    proiT2 = dti("proiT2", [BL, 128, ACH, N], PROI)
    hT = dti("hT", [RNN, BL], SML)
    ctxT = dti("ctxT", [RNN, BL], F32)
    wvp = dti("wvp", [RNN, ATT], BIG)
    wh4t = dti("wh4t", [4, 2, 2, 128, KT // 2, 512], SML)
    wv2t = dti("wv2t", [2, 2, 2, 128, KT // 2, 512], SML)
    gluwt = dti("gluwt", [2, 8, 2, 128, KT // 2, 512], GLUDT)
    wa4 = dti("wa4", [4, ATT], F32R)
    wb4 = dti("wb4", [4, ATT], F32)
    wvb3 = dti("wvb3", [3, ATT], F32)
    glub = dti("glub", [1, 2 * RNN], GLUDT)
    mp = dti("mp", [1, 2 * BL], F32)

    xs_o = dto("xs", [BL, RNN], F32)       # [16, 2048] x rows
    on_o = dto("onat", [BL, RNN], F32)     # [16, 2048] output rows
    gw_o = dto("gw", [BL, N], F32)
    cw_o = dto("cw", [BL, 2], F32)
    pw_o = dto("pw", [BL, N], F32)
    ow_o = dto("ow", [BL, 2], F32)

    with tile.TileContext(nc) as tc, ExitStack() as ctx:
        const = ctx.enter_context(tc.tile_pool(name="const", bufs=1))
        wstr = ctx.enter_context(tc.tile_pool(name="wstr", bufs=2))
        rstr = ctx.enter_context(tc.tile_pool(name="rstr", bufs=2))
        pstr = ctx.enter_context(tc.tile_pool(name="pstr", bufs=3))
        work = ctx.enter_context(tc.tile_pool(name="work", bufs=2))
        ttp = ctx.enter_context(tc.tile_pool(name="ttp", bufs=3))
        pmm = ctx.enter_context(tc.tile_pool(name="pmm", bufs=2, space="PSUM"))
        pvec = ctx.enter_context(tc.tile_pool(name="pvec", bufs=3, space="PSUM"))
        ptr = ctx.enter_context(tc.tile_pool(name="ptr", bufs=2, space="PSUM"))
        dram = ctx.enter_context(tc.tile_pool(name="dram", bufs=1, space="DRAM"))

        # ---- constants ----
        ident = const.tile([128, 128], F32, tag="ident")
        make_identity(nc, ident)
        ones_g = const.tile([1, BL], GLUDT, tag="ones_g")
        nc.vector.memset(ones_g, 1.0)
        ones_c = const.tile([1, 128], F32, tag="ones_c")
        nc.vector.memset(ones_c, 1.0)

        wa_sb = const.tile([128, 4, ACH], F32R, tag="wa_sb")
        nc.sync.dma_start(wa_sb, wa4.rearrange("j (ch p) -> p j ch", p=128))
        wb_sb = const.tile([128, 4, ACH], F32, tag="wb_sb")
        nc.sync.dma_start(wb_sb, wb4.rearrange("j (ch p) -> p j ch", p=128))
        wvb_sb = const.tile([128, 3, ACH], F32, tag="wvb_sb")
        nc.sync.dma_start(wvb_sb, wvb3.rearrange("j (ch p) -> p j ch", p=128))
        mp_sb = const.tile([1, 2 * BL], F32, tag="mp_sb")
        nc.sync.dma_start(mp_sb, mp)

        hT_sb = const.tile([128, KT, BL], SML, tag="hT_sb")
        nc.sync.dma_start(hT_sb, hT.rearrange("(kt p) b -> p kt b", p=128))
        ctxT_sb = const.tile([128, KT, BL], F32, tag="ctxT_sb")
        nc.sync.dma_start(ctxT_sb, ctxT.rearrange("(kt p) b -> p kt b", p=128))
        wvp_sb = const.tile([128, KT, ATT], BIG, tag="wvp_sb")
        nc.sync.dma_start(wvp_sb, wvp.rearrange("(kt p) m -> p kt m", p=128))

        # persistent state
        hT4 = const.tile([128, 4, ACH, BL], F32, tag="hT4")
        globT = const.tile([128, KT, BL], F32, tag="globT")
        ctxoT = const.tile([128, KT, BL], F32, tag="ctxoT")
        hpc = const.tile([128, ACH, BL], F32, tag="hpc")
        compT = const.tile([128, KT, BL], F32, tag="compT")
        outT = const.tile([128, KT, BL], F32, tag="outT")
        scALL = const.tile([1, NPAIR, GRP], F32, tag="scALL")

        RTg = [
            dram.tile([ACH, 128, GRP], BF16, tag=f"RTg{g}", name=f"RTg{g}")
            for g in range(NGRP)
        ]
        zh_d = dram.tile([BL, 2 * RNN], F32, tag="zh_d")

        gwT0 = const.tile([128, BL], EIN, tag="gwT0")
        gwT1 = const.tile([128, BL], EIN, tag="gwT1")
        pwT0 = const.tile([128, BL], EIN, tag="pwT0")
        pwT1 = const.tile([128, BL], EIN, tag="pwT1")

        # fused psum->sbuf transpose helper (DVE copyback; keeps ACT free)
        def transpose_cb(in_sb, out_ap, bias=None, extra_add=None):
            p_in = in_sb.shape[0]
            f_in = in_sb.shape[1]
            tr = ptr.tile([128, 128], F32, tag="ptr")
            trv = tr[:f_in, :p_in]
            nc.tensor.transpose(trv, in_sb, ident[:p_in, :p_in])
            if extra_add is not None:
                if bias is not None:
                    tmp = ttp.tile([128, BL], F32, tag="trtmp")
                    tv = tmp[:f_in, :p_in]
                    nc.vector.tensor_scalar_add(tv, trv, bias[:f_in, :])
                    nc.vector.tensor_add(out_ap, tv, extra_add)
                else:
                    nc.vector.tensor_add(out_ap, trv, extra_add)
            elif bias is not None:
                nc.vector.tensor_scalar_add(out_ap, trv, bias[:f_in, :])
            else:
                nc.vector.tensor_copy(out_ap, trv)

        # ---- S1: h projections (hg,hc,hp,ho) -> hT4[:, j, ch, :] ----
        for j in range(4):
            hn = work.tile([2 * BL, ATT], F32, tag="projnat")
            for ng in range(2):
                ps = pmm.tile([BL, 512], F32, tag="pmm")
                for kh in range(2):
                    wt = wstr.tile([128, KT // 2, 512], SML, tag="wst")
                    nc.sync.dma_start(wt, wh4t[j, ng, kh])
                    for k8 in range(KT // 2):
                        kt = kh * 8 + k8
                        nc.tensor.matmul(
                            ps, hT_sb[:, kt, :], wt[:, k8, :],
                            start=(kt == 0), stop=(kt == KT - 1),
                        )
                nc.vector.tensor_copy(hn[:BL, ng * 512 : (ng + 1) * 512], ps)
            for ch in range(ACH):
                transpose_cb(
                    hn[:BL, ch * 128 : (ch + 1) * 128],
                    hT4[:, j, ch, :],
                    bias=wb_sb[:, j, ch : ch + 1],
                )

        # batched softmax over all BL rows stashed in scALL
        def softmax_batch(w_o, wT0, wT1):
            for b in range(BL):
                sl = scALL[:, b // 2, (b % 2) * N : (b % 2) * N + N]
                nm = ttp.tile([1, 1], F32, tag="nm")
                nc.vector.tensor_reduce(
                    out=nm, in_=sl, op=ALU.max, axis=AXX, negate=True
                )
                ex = ttp.tile([1, N], F32, tag="ex", bufs=2)
                se = ttp.tile([1, 1], F32, tag="se")
                nc.scalar.activation(
                    ex, sl, AF.Exp, bias=nm, scale=1.0, accum_out=se
                )
                rcp = ttp.tile([1, 1], F32, tag="rc")
                nc.vector.reciprocal(rcp, se)
                wf = ttp.tile([1, N], F32, tag="wf", bufs=2)
                nc.vector.tensor_scalar_mul(wf, ex, rcp)
                nc.gpsimd.dma_start(w_o[b : b + 1, :], wf)
                tra = ptr.tile([128, 128], F32, tag="ptr")
                nc.tensor.transpose(tra[:128, :1], wf[:, :128], ident[:1, :1])
                nc.vector.tensor_copy(wT0[:, b : b + 1], tra[:128, :1])
                trb = ptr.tile([128, 128], F32, tag="ptr")
                nc.tensor.transpose(trb[: N - 128, :1], wf[:, 128:N], ident[:1, :1])
                nc.vector.tensor_copy(wT1[: N - 128, b : b + 1], trb[: N - 128, :1])

        # ---- S2: global attention scores -> gw ----
        for pb in range(NPAIR):
            pts = []
            for h in range(2):
                pt = pstr.tile([128, ACH, N], PROI, tag="pt")
                nc.scalar.dma_start(pt, proiT2[2 * pb + h])
                pts.append(pt)
            ps_s = pvec.tile([1, GRP], F32, tag="pv")
            for ch in range(ACH):
                tt = ttp.tile([128, GRP], F32R, tag="tt", bufs=2)
                for h in range(2):
                    b = 2 * pb + h
                    nc.scalar.activation(
                        tt[:, h * N : (h + 1) * N], pts[h][:, ch, :], AF.Tanh,
                        bias=hT4[:, 0, ch, b : b + 1], scale=1.0,
                    )
                nc.tensor.matmul(
                    ps_s, wa_sb[:, 0, ch : ch + 1], tt,
                    start=(ch == 0), stop=(ch == ACH - 1),
                )
            nc.vector.tensor_copy(scALL[:, pb, :], ps_s)
        softmax_batch(gw_o, gwT0, gwT1)

        # ---- S3: big matmul R^T = wvp^T @ roiT -> per-group DRAM (bf16) ----
        for g in range(NGRP):
            rt = rstr.tile([128, KT, GRP], BIG, tag="rt")
            nc.sync.dma_start(rt, roiTt[g])
            for m in range(ACH):
                ps = pmm.tile([128, GRP], F32, tag="pmm")
                for kt in range(KT):
                    nc.tensor.matmul(
                        ps, wvp_sb[:, kt, m * 128 : (m + 1) * 128], rt[:, kt, :],
                        start=(kt == 0), stop=(kt == KT - 1),
                    )
                rc_sb = work.tile([128, GRP], BF16, tag="rtcb")
                nc.vector.tensor_copy(rc_sb, ps)
                nc.sync.dma_start(RTg[g][m], rc_sb)

        # ---- S10a: GLU h-half (z_h = h @ glu_w[RNN:, :]) -> DRAM spill ----
        for ng in range(8):
            ps = pmm.tile([BL, 512], F32, tag="pmm")
            for kh in range(2):
                gt = wstr.tile([128, KT // 2, 512], GLUDT, tag="wst")
                nc.sync.dma_start(gt, gluwt[1, ng, kh])
                for k8 in range(KT // 2):
                    kt = kh * 8 + k8
                    nc.tensor.matmul(
                        ps, hT_sb[:, kt, :], gt[:, k8, :],
                        start=(kt == 0), stop=(kt == KT - 1),
                    )
            zt = work.tile([BL, 512], F32, tag="zt")
            nc.vector.tensor_copy(zt, ps)
            nc.sync.dma_start(zh_d[:, ng * 512 : (ng + 1) * 512], zt)

        # ---- einsum: out_nat[b, :] = sum_n w[b, n] * roi[b, n, :] ----
        def einsum_rows(wT0, wT1, nat_out):
            for b in range(BL):
                ra = rstr.tile([128, RNN], EIN, tag="ra")
                nc.scalar.dma_start(ra, roinat[b * N : b * N + 128, :])
                rb = rstr.tile([128, RNN], EIN, tag="rb")
                nc.scalar.dma_start(
                    rb[: N - 128, :], roinat[b * N + 128 : (b + 1) * N, :]
                )
                for dg in range(4):
                    pe = pvec.tile([1, 512], F32, tag="pv")
                    nc.tensor.matmul(
                        pe, wT0[:, b : b + 1], ra[:, dg * 512 : (dg + 1) * 512],
                        start=True, stop=False,
                    )
                    nc.tensor.matmul(
                        pe, wT1[: N - 128, b : b + 1],
                        rb[: N - 128, dg * 512 : (dg + 1) * 512],
                        start=False, stop=True,
                    )
                    grow = work.tile([1, 512], F32, tag="grow")
                    nc.vector.tensor_copy(grow, pe)
                    nc.gpsimd.dma_start(
                        nat_out[b : b + 1, dg * 512 : (dg + 1) * 512], grow
                    )

        # ---- S4: global_out ----
        gnat = work.tile([BL, RNN], F32, tag="natrow")
        einsum_rows(gwT0, gwT1, gnat)
        for kt in range(KT):
            transpose_cb(gnat[:, kt * 128 : (kt + 1) * 128], globT[:, kt, :])

        # ---- S5: context attention ----
        r2 = const.tile([128, KT, 2 * BL], SML, tag="r2")
        for kt in range(KT):
            r2v = r2[:, kt, :].rearrange("p (b k) -> p b k", k=2)
            nc.vector.tensor_copy(r2v[:, :, 0], globT[:, kt, :])
            nc.vector.tensor_copy(r2v[:, :, 1], ctxT_sb[:, kt, :])

        def kv2_attention(stat_sb, wj, vbj, hj, score_j, mask_sb, w_out_dram):
            """2-key attention over stat_sb [128, KT, 32] (d x (b,k))."""
            vnat = work.tile([2 * BL, ATT], F32, tag="projnat")
            for ng in range(2):
                ps = pmm.tile([2 * BL, 512], F32, tag="pmm")
                for kh in range(2):
                    wt = wstr.tile([128, KT // 2, 512], SML, tag="wst")
                    nc.sync.dma_start(wt, wv2t[wj, ng, kh])
                    for k8 in range(KT // 2):
                        kt = kh * 8 + k8
                        nc.tensor.matmul(
                            ps, stat_sb[:, kt, :], wt[:, k8, :],
                            start=(kt == 0), stop=(kt == KT - 1),
                        )
                nc.vector.tensor_copy(vnat[:, ng * 512 : (ng + 1) * 512], ps)
            ps_c = pvec.tile([1, 2 * BL], F32, tag="pv")
            for ch in range(ACH):
                vT = ttp.tile([128, 2 * BL], F32, tag="vT")
                transpose_cb(
                    vnat[:, ch * 128 : (ch + 1) * 128], vT,
                    bias=wvb_sb[:, vbj, ch : ch + 1],
                )
                ta = ttp.tile([128, 2 * BL], F32, tag="ta")
                nc.vector.tensor_add(
                    ta.rearrange("p (b k) -> p b k", k=2),
                    vT.rearrange("p (b k) -> p b k", k=2),
                    hT4[:, hj, ch, :].unsqueeze(2).broadcast_to([128, BL, 2]),
                )
                tb = ttp.tile([128, 2 * BL], F32R, tag="tb")
                nc.scalar.activation(tb, ta, AF.Tanh)
                nc.tensor.matmul(
                    ps_c, wa_sb[:, score_j, ch : ch + 1], tb,
                    start=(ch == 0), stop=(ch == ACH - 1),
                )
            ew = ttp.tile([1, 2 * BL], F32, tag="ew")
            nc.scalar.activation(ew, ps_c, AF.Exp)
            if mask_sb is not None:
                mw = ttp.tile([1, 2 * BL], F32, tag="mw")
                nc.vector.tensor_mul(mw, ew, mask_sb)
                ew = mw
            ssum = ttp.tile([1, BL], F32, tag="ssum")
            nc.vector.tensor_reduce(
                out=ssum.unsqueeze(2),
                in_=ew.rearrange("p (b k) -> p b k", k=2),
                op=ALU.add, axis=AXX,
            )
            rcp = ttp.tile([1, BL], F32, tag="rc16")
            nc.vector.reciprocal(rcp, ssum)
            wgt = ttp.tile([1, 2 * BL], F32, tag="wgt")
            nc.vector.tensor_mul(
                wgt.rearrange("p (b k) -> p b k", k=2),
                ew.rearrange("p (b k) -> p b k", k=2),
                rcp.unsqueeze(2).broadcast_to([1, BL, 2]),
            )
            nc.gpsimd.dma_start(
                w_out_dram.rearrange("b k -> (b k)").unsqueeze(0), wgt
            )
            psr = ptr.tile([128, 128], F32, tag="ptr")
            nc.tensor.matmul(
                psr[:, : 2 * BL], ones_c[:, :128], wgt, start=True, stop=True,
            )
            wr = work.tile([128, 2 * BL], F32, tag="wrep")
            nc.vector.tensor_copy(wr, psr[:, : 2 * BL])
            return wr

        cwr = kv2_attention(r2, 0, 0, 1, 1, mp_sb, cw_o)
        cwv = cwr.rearrange("p (b k) -> p b k", k=2)
        for kt in range(KT):
            t0 = ttp.tile([128, BL], F32, tag="cmb0")
            nc.vector.tensor_mul(t0, globT[:, kt, :], cwv[:, :, 0])
            t1 = ttp.tile([128, BL], F32, tag="cmb1")
            nc.vector.tensor_mul(t1, ctxT_sb[:, kt, :], cwv[:, :, 1])
            nc.vector.tensor_add(ctxoT[:, kt, :], t0, t1)

        # ---- S6: hpc = hp + (context_out @ wvp + wvp_b)^T ----
        cxb = const.tile([128, KT, BL], BIG, tag="cxb")
        for kt in range(KT):
            nc.vector.tensor_copy(cxb[:, kt, :], ctxoT[:, kt, :])
        cvpn = work.tile([2 * BL, ATT], F32, tag="projnat")
        for ng in range(2):
            ps = pmm.tile([BL, 512], F32, tag="pmm")
            for kt in range(KT):
                nc.tensor.matmul(
                    ps, cxb[:, kt, :], wvp_sb[:, kt, ng * 512 : (ng + 1) * 512],
                    start=(kt == 0), stop=(kt == KT - 1),
                )
            nc.vector.tensor_copy(cvpn[:BL, ng * 512 : (ng + 1) * 512], ps)
        for ch in range(ACH):
            transpose_cb(
                cvpn[:BL, ch * 128 : (ch + 1) * 128], hpc[:, ch, :],
                bias=wvb_sb[:, 1, ch : ch + 1], extra_add=hT4[:, 2, ch, :],
            )

        # ---- S7: composition scores -> pw (per-pair, overlaps S3) ----
        for pb in range(NPAIR):
            ps_s = pvec.tile([1, GRP], F32, tag="pv")
            for ch in range(ACH):
                rtt = pstr.tile([128, GRP], BF16, tag="rtt")
                nc.scalar.dma_start(rtt, RTg[pb][ch])
                tt = ttp.tile([128, GRP], F32R, tag="tt", bufs=2)
                for h in range(2):
                    b = 2 * pb + h
                    nc.scalar.activation(
                        tt[:, h * N : (h + 1) * N], rtt[:, h * N : (h + 1) * N],
                        AF.Tanh, bias=hpc[:, ch, b : b + 1], scale=-1.0,
                    )
                nc.tensor.matmul(
                    ps_s, wa_sb[:, 2, ch : ch + 1], tt,
                    start=(ch == 0), stop=(ch == ACH - 1),
                )
            nc.vector.tensor_copy(scALL[:, pb, :], ps_s)
        softmax_batch(pw_o, pwT0, pwT1)

        # ---- S8: comp_out^T = ctxo^T - sum_n pw*roi ----
        cnat = work.tile([BL, RNN], F32, tag="natrow")
        einsum_rows(pwT0, pwT1, cnat)
        for kt in range(KT):
            tr = ptr.tile([128, 128], F32, tag="ptr")
            nc.tensor.transpose(
                tr[:, :BL], cnat[:, kt * 128 : (kt + 1) * 128], ident[:BL, :BL]
            )
            nc.vector.tensor_sub(compT[:, kt, :], ctxoT[:, kt, :], tr[:, :BL])

        # ---- S9: output attention ----
        of2 = const.tile([128, KT, 2 * BL], SML, tag="of2")
        for kt in range(KT):
            ofv = of2[:, kt, :].rearrange("p (b k) -> p b k", k=2)
            nc.vector.tensor_copy(ofv[:, :, 0], globT[:, kt, :])
            nc.vector.tensor_copy(ofv[:, :, 1], compT[:, kt, :])
        owr = kv2_attention(of2, 1, 2, 3, 3, None, ow_o)
        owv = owr.rearrange("p (b k) -> p b k", k=2)
        for kt in range(KT):
            t0 = ttp.tile([128, BL], F32, tag="cmb0")
            nc.vector.tensor_mul(t0, globT[:, kt, :], owv[:, :, 0])
            t1 = ttp.tile([128, BL], F32, tag="cmb1")
            nc.vector.tensor_mul(t1, compT[:, kt, :], owv[:, :, 1])
            nc.vector.tensor_add(outT[:, kt, :], t0, t1)

        # output rows (natural layout) to DRAM
        onat = work.tile([BL, RNN], F32, tag="onat")
        for kt in range(KT):
            tr = ptr.tile([128, 128], F32, tag="ptr")
            nc.tensor.transpose(tr[:BL, :], outT[:, kt, :], ident[:128, :128])
            nc.vector.tensor_copy(onat[:, kt * 128 : (kt + 1) * 128], tr[:BL, :])
        nc.sync.dma_start(on_o, onat)

        # ---- S10b: GLU output-half + gate ----
        outTg = const.tile([128, KT, BL], GLUDT, tag="outTg")
        for kt in range(KT):
            nc.vector.tensor_copy(outTg[:, kt, :], outT[:, kt, :])
        for ngx in range(4):
            za = work.tile([BL, 512], F32, tag="za")
            zb = work.tile([BL, 512], F32, tag="zb")
            for half, zdst in ((0, za), (1, zb)):
                ng = ngx + 4 * half
                ps = pmm.tile([BL, 512], F32, tag="pmm")
                for kh in range(2):
                    gt = wstr.tile([128, KT // 2, 512], GLUDT, tag="wst")
                    nc.sync.dma_start(gt, gluwt[0, ng, kh])
                    for k8 in range(KT // 2):
                        kt = kh * 8 + k8
                        nc.tensor.matmul(
                            ps, outTg[:, kt, :], gt[:, k8, :],
                            start=(kt == 0), stop=False,
                        )
                gb = wstr.tile([1, 512], GLUDT, tag="gbt")
                nc.sync.dma_start(gb, glub[:, ng * 512 : (ng + 1) * 512])
                nc.tensor.matmul(ps, ones_g, gb, start=False, stop=True)
                zhp = work.tile([BL, 512], F32, tag="zhp")
                nc.sync.dma_start(zhp, zh_d[:, ng * 512 : (ng + 1) * 512])
                nc.vector.tensor_add(zdst, ps, zhp)
            sg = work.tile([BL, 512], F32, tag="sg")
            nc.scalar.activation(sg, zb, AF.Sigmoid)
            xt = work.tile([BL, 512], F32, tag="xt")
            nc.vector.tensor_mul(xt, za, sg)
            nc.sync.dma_start(xs_o[:, ngx * 512 : (ngx + 1) * 512], xt)

    nc.compile()
    return nc


_NC_CACHE = None


def _get_nc():
    global _NC_CACHE
    if _NC_CACHE is None:
        _NC_CACHE = build()
    return _NC_CACHE


def _tile_w(w):
    """[2048, 1024] -> [2(ng), 2(kh), 128(p), 8(k8), 512(c)] DMA-tiled."""
    a = w.reshape(2, 8, 128, 2, 512)  # [kh, k8, p, ng, c]
    return a.transpose(3, 0, 2, 1, 4)  # [ng, kh, p, k8, c]


def prep_inputs(
    h, roi_feats, p_roi_feats, mask, context,
    whg_w, whg_b, wag_w, wag_b,
    whc_w, whc_b, wvc_w, wvc_b, wac_w, wac_b,
    whp_w, whp_b, wvp_w, wvp_b, wap_w, wap_b,
    wvo_w, wvo_b, who_w, who_b, wao_w, wao_b,
    glu_w, glu_b,
):
    """Build the 8 per-core input maps (host-side sharding + layout)."""
    h = np.asarray(h, np.float32)
    roi = np.asarray(roi_feats, np.float32)
    proi = np.asarray(p_roi_feats, np.float32)
    mask = np.asarray(mask)
    context = np.asarray(context, np.float32)

    wh4t = _cast(
        np.stack([_tile_w(np.asarray(w)) for w in (whg_w, whc_w, whp_w, who_w)]),
        SML,
    )
    wv2t = _cast(np.stack([_tile_w(np.asarray(w)) for w in (wvc_w, wvo_w)]), SML)
    wa4 = _cast(np.stack([wag_w, wac_w, wap_w, wao_w]), F32R)
    wb4 = _cast(np.stack([whg_b, whc_b, whp_b, who_b]), F32)
    wvb3 = _cast(np.stack([wvc_b, wvp_b, wvo_b]), F32)
    wvp = _cast(wvp_w, BIG)
    # gluwt[half, ng, kh, p, k8, c]; half 0 = output rows (0:2048)
    glw = np.asarray(glu_w, np.float32)
    gluwt = _cast(
        glw.reshape(2, 2, 8, 128, 8, 512).transpose(0, 4, 1, 3, 2, 5), GLUDT
    )
    glub = _cast(np.asarray(glu_b, np.float32).reshape(1, -1), GLUDT)

    # NOTE: wag_b/wac_b/wap_b/wao_b shift scores by a constant, which cancels
    # in the softmax (also under the mask-renormalization), so they are unused.

    in_maps = []
    for c in range(NCORES):
        rows = slice(c * BL, (c + 1) * BL)
        roi_bf = _cast(roi[rows].reshape(ROWS, RNN), EIN)
        # [NGRP, 128(p), KT, GRP]: A[g, p, kt, r] = roi_c[g*GRP + r, kt*128 + p]
        roiTt = np.ascontiguousarray(
            roi_bf.reshape(NGRP, GRP, KT, 128).transpose(0, 3, 2, 1)
        )
        # proiT2[b, p, ch, n] = proi[b, n, ch*128 + p]
        proiT2 = np.ascontiguousarray(
            _cast(proi[rows], PROI).reshape(BL, N, ACH, 128).transpose(0, 3, 2, 1)
        )
        cm = (mask[rows] > 0).astype(np.float32)
        mpv = np.stack([np.ones(BL, np.float32), cm], axis=1).reshape(1, 2 * BL)
        in_maps.append(
            dict(
                roiTt=roiTt,
                roinat=roi_bf,
                proiT2=proiT2,
                hT=_cast(h[rows].T, SML),
                ctxT=_cast(context[rows].T, F32),
                wvp=wvp, wh4t=wh4t, wv2t=wv2t, wa4=wa4, wb4=wb4, wvb3=wvb3,
                gluwt=gluwt, glub=glub, mp=mpv,
            )
        )
    return in_maps


def run_on_device(in_maps, trace=False):
    nc = _get_nc()
    return run_bass_kernel_spmd(nc, in_maps, list(range(NCORES)), trace=trace)


def assemble(results):
    x = np.concatenate([r["xs"] for r in results], axis=0).astype(np.float32)
    output = np.concatenate([r["onat"] for r in results], axis=0).astype(np.float32)
    gw = np.concatenate([r["gw"] for r in results], axis=0).astype(np.float32)
    cw = np.concatenate([r["cw"] for r in results], axis=0).astype(np.float32)
    pw = np.concatenate([r["pw"] for r in results], axis=0).astype(np.float32)
    ow = np.concatenate([r["ow"] for r in results], axis=0).astype(np.float32)
    return (x, output, gw, cw, pw, ow)


def kernel(**inputs):
    in_maps = prep_inputs(**inputs)
    res = run_on_device(in_maps, trace=False)
    return assemble(res.results)


# revision 10
# speedup vs baseline: 1.2302x; 1.0152x over previous
"""Trainium2 Bass kernel for nn_Attention_9457517985916.

Multi-stage attention (Global/Context/Composition/Output) + GLU fusion.
B=128, N=196, RNN=2048, ATT=1024 on 8 NeuronCores, data-parallel over B
(16 rows per core, weights replicated).

Layout strategy: every bulk stream is pre-tiled on the host so each DMA
lands in SBUF with maximal-contiguity descriptors (KB-scale per-partition
runs instead of the 392..784B runs a strided read would produce):

  - roiTt  [8, 128, 16, 392]      per-group d-major roi rows (big matmul)
  - roinat [3136, 2048]           n-major rows (attention-weighted sums)
  - proiT2 [16, 128, 8, 196]      att-major p_roi (global score pass)
  - wh4t/wv2t/gluwt               weight chunks pre-arranged per DMA tile

Compute structure per core:
  R^T = wvp^T @ roiT (bf16, fp32 accumulate) spills per-group to DRAM;
  score passes are ACT tanh(in*scale + bias_col) followed by float32r
  matmuls against the wa vectors; softmaxes are batched to avoid ACT
  table reloads; the attention-weighted row sums run as (196->2)-chunk
  PE matmuls over natural-layout roi; GLU runs in two halves (h-half
  early, output-half at the tail). No collectives; every graded output
  is a per-core row slice the host reassembles.
"""

from contextlib import ExitStack

import numpy as np
import ml_dtypes

import concourse.bass as bass
import concourse.tile as tile
from concourse import bacc, mybir
from concourse.bass_utils import run_bass_kernel_spmd
from concourse.masks import make_identity

F32 = mybir.dt.float32
BF16 = mybir.dt.bfloat16
F32R = mybir.dt.float32r
AF = mybir.ActivationFunctionType
ALU = mybir.AluOpType
AXX = mybir.AxisListType.X

B, N, RNN, ATT = 128, 196, 2048, 1024
NCORES = 8
BL = B // NCORES          # 16 rows per core
ROWS = BL * N             # 3136
KT = RNN // 128           # 16 d-chunks
ACH = ATT // 128          # 8 att-chunks
NPAIR = BL // 2           # 8 row pairs
GRP = 2 * N               # 392 big-matmul rows per group (2 batch rows)
NGRP = ROWS // GRP        # 8

# dtype knobs
BIG = BF16     # roiT / wvp / big matmul + cvp
EIN = BF16     # roinat / einsum matmuls
PROI = BF16    # p_roi stream
SML = BF16     # wh4 / wv2 / hT projections
GLUDT = BF16   # glu weights + glu matmuls

_NP = {F32: np.float32, F32R: np.float32, BF16: ml_dtypes.bfloat16}


def _cast(a, dt):
    return np.ascontiguousarray(np.asarray(a), dtype=_NP[dt])


def build():
    nc = bacc.Bacc("TRN2", target_bir_lowering=False, debug=False)
    dti = lambda n, s, d: nc.dram_tensor(n, s, d, kind="ExternalInput").ap()
    dto = lambda n, s, d: nc.dram_tensor(n, s, d, kind="ExternalOutput").ap()

    roiTt = dti("roiTt", [NGRP, 128, KT, GRP], BIG)
    roinat = dti("roinat", [ROWS, RNN], EIN)
    proiT2 = dti("proiT2", [BL, 128, ACH, N], PROI)
    hT = dti("hT", [RNN, BL], SML)
    ctxT = dti("ctxT", [RNN, BL], F32)
    wvp = dti("wvp", [RNN, ATT], BIG)
    wh4t = dti("wh4t", [4, 2, 2, 128, KT // 2, 512], SML)
    wv2t = dti("wv2t", [2, 2, 2, 128, KT // 2, 512], SML)
    gluwt = dti("gluwt", [2, 8, 2, 128, KT // 2, 512], GLUDT)
    wa4 = dti("wa4", [4, ATT], BF16)
    wb4 = dti("wb4", [4, ATT], F32)
    wvb3 = dti("wvb3", [3, ATT], F32)
    glub = dti("glub", [1, 2 * RNN], GLUDT)
    mp = dti("mp", [1, 2 * BL], F32)

    xs_o = dto("xs", [BL, RNN], F32)       # [16, 2048] x rows
    on_o = dto("onat", [BL, RNN], F32)     # [16, 2048] output rows
    gw_o = dto("gw", [BL, N], F32)
    cw_o = dto("cw", [BL, 2], F32)
    pw_o = dto("pw", [BL, N], F32)
    ow_o = dto("ow", [BL, 2], F32)

    with tile.TileContext(nc) as tc, ExitStack() as ctx:
        const = ctx.enter_context(tc.tile_pool(name="const", bufs=1))
        wstr = ctx.enter_context(tc.tile_pool(name="wstr", bufs=2))
        rstr = ctx.enter_context(tc.tile_pool(name="rstr", bufs=2))
        pstr = ctx.enter_context(tc.tile_pool(name="pstr", bufs=3))
        work = ctx.enter_context(tc.tile_pool(name="work", bufs=2))
        ttp = ctx.enter_context(tc.tile_pool(name="ttp", bufs=3))
        pmm = ctx.enter_context(tc.tile_pool(name="pmm", bufs=2, space="PSUM"))
        pvec = ctx.enter_context(tc.tile_pool(name="pvec", bufs=3, space="PSUM"))
        ptr = ctx.enter_context(tc.tile_pool(name="ptr", bufs=2, space="PSUM"))
        dram = ctx.enter_context(tc.tile_pool(name="dram", bufs=1, space="DRAM"))

        # ---- constants ----
        ident = const.tile([128, 128], F32, tag="ident")
        make_identity(nc, ident)
        ones_g = const.tile([1, BL], GLUDT, tag="ones_g")
        nc.vector.memset(ones_g, 1.0)
        ones_c = const.tile([1, 128], F32, tag="ones_c")
        nc.vector.memset(ones_c, 1.0)

        wa_sb = const.tile([128, 4, ACH], BF16, tag="wa_sb")
        nc.sync.dma_start(wa_sb, wa4.rearrange("j (ch p) -> p j ch", p=128))
        wb_sb = const.tile([128, 4, ACH], F32, tag="wb_sb")
        nc.sync.dma_start(wb_sb, wb4.rearrange("j (ch p) -> p j ch", p=128))
        wvb_sb = const.tile([128, 3, ACH], F32, tag="wvb_sb")
        nc.sync.dma_start(wvb_sb, wvb3.rearrange("j (ch p) -> p j ch", p=128))
        mp_sb = const.tile([1, 2 * BL], F32, tag="mp_sb")
        nc.sync.dma_start(mp_sb, mp)

        hT_sb = const.tile([128, KT, BL], SML, tag="hT_sb")
        nc.sync.dma_start(hT_sb, hT.rearrange("(kt p) b -> p kt b", p=128))
        ctxT_sb = const.tile([128, KT, BL], F32, tag="ctxT_sb")
        nc.sync.dma_start(ctxT_sb, ctxT.rearrange("(kt p) b -> p kt b", p=128))
        wvp_sb = const.tile([128, KT, ATT], BIG, tag="wvp_sb")
        nc.sync.dma_start(wvp_sb, wvp.rearrange("(kt p) m -> p kt m", p=128))

        # persistent state
        hT4 = const.tile([128, 4, ACH, BL], F32, tag="hT4")
        globT = const.tile([128, KT, BL], F32, tag="globT")
        ctxoT = const.tile([128, KT, BL], F32, tag="ctxoT")
        hpc = const.tile([128, ACH, BL], F32, tag="hpc")
        compT = const.tile([128, KT, BL], F32, tag="compT")
        outT = const.tile([128, KT, BL], F32, tag="outT")

        RTg = [
            dram.tile([ACH, 128, GRP], BF16, tag=f"RTg{g}", name=f"RTg{g}")
            for g in range(NGRP)
        ]
        zh_d = dram.tile([BL, 2 * RNN], F32, tag="zh_d")

        gwT0 = const.tile([128, BL], EIN, tag="gwT0")
        gwT1 = const.tile([128, BL], EIN, tag="gwT1")
        pwT0 = const.tile([128, BL], EIN, tag="pwT0")
        pwT1 = const.tile([128, BL], EIN, tag="pwT1")

        # fused psum->sbuf transpose helper (DVE copyback; keeps ACT free)
        def transpose_cb(in_sb, out_ap, bias=None, extra_add=None):
            p_in = in_sb.shape[0]
            f_in = in_sb.shape[1]
            tr = ptr.tile([128, 128], F32, tag="ptr")
            trv = tr[:f_in, :p_in]
            nc.tensor.transpose(trv, in_sb, ident[:p_in, :p_in])
            if extra_add is not None:
                if bias is not None:
                    tmp = ttp.tile([128, BL], F32, tag="trtmp")
                    tv = tmp[:f_in, :p_in]
                    nc.vector.tensor_scalar_add(tv, trv, bias[:f_in, :])
                    nc.vector.tensor_add(out_ap, tv, extra_add)
                else:
                    nc.vector.tensor_add(out_ap, trv, extra_add)
            elif bias is not None:
                nc.vector.tensor_scalar_add(out_ap, trv, bias[:f_in, :])
            else:
                nc.vector.tensor_copy(out_ap, trv)

        # ---- h projection j -> hT4[:, j, ch, :] ----
        def hproj(j):
            hn = work.tile([2 * BL, ATT], F32, tag="projnat", name="hn")
            for ng in range(2):
                ps = pmm.tile([BL, 512], F32, tag="pmm", name="ps_h")
                for kh in range(2):
                    wt = wstr.tile(
                        [128, KT // 2, 512], SML, tag="wst", name="wt_h"
                    )
                    nc.sync.dma_start(wt, wh4t[j, ng, kh])
                    for k8 in range(KT // 2):
                        kt = kh * 8 + k8
                        nc.tensor.matmul(
                            ps, hT_sb[:, kt, :], wt[:, k8, :],
                            start=(kt == 0), stop=(kt == KT - 1),
                        )
                nc.vector.tensor_copy(hn[:BL, ng * 512 : (ng + 1) * 512], ps)
            for ch in range(ACH):
                transpose_cb(
                    hn[:BL, ch * 128 : (ch + 1) * 128],
                    hT4[:, j, ch, :],
                    bias=wb_sb[:, j, ch : ch + 1],
                )

        # softmax of one [1, N] psum slice -> weight row + transposed columns
        def softmax_row(sl, w_row, wT0, wT1, b):
            nm = ttp.tile([1, 1], F32, tag="nm", name="nm")
            nc.vector.tensor_reduce(
                out=nm, in_=sl, op=ALU.max, axis=AXX, negate=True
            )
            ex = ttp.tile([1, N], F32, tag="ex", bufs=2, name="ex")
            se = ttp.tile([1, 1], F32, tag="se", name="se")
            nc.scalar.activation(ex, sl, AF.Exp, bias=nm, scale=1.0, accum_out=se)
            rcp = ttp.tile([1, 1], F32, tag="rc", name="rcp")
            nc.vector.reciprocal(rcp, se)
            wf = ttp.tile([1, N], F32, tag="wf", bufs=2, name="wf")
            nc.vector.tensor_scalar_mul(wf, ex, rcp)
            nc.gpsimd.dma_start(w_row, wf)
            tra = ptr.tile([128, 128], F32, tag="ptr", name="tra")
            nc.tensor.transpose(tra[:128, :1], wf[:, :128], ident[:1, :1])
            nc.vector.tensor_copy(wT0[:, b : b + 1], tra[:128, :1])
            trb = ptr.tile([128, 128], F32, tag="ptr", name="trb")
            nc.tensor.transpose(trb[: N - 128, :1], wf[:, 128:N], ident[:1, :1])
            nc.vector.tensor_copy(wT1[: N - 128, b : b + 1], trb[: N - 128, :1])

        # einsum: nat_out[b, :] = sum_n w[b, n] * roi[b, n, :]
        def einsum_b(b, wT0, wT1, nat_out):
            ra = rstr.tile([128, RNN], EIN, tag="ra", name="ra")
            nc.scalar.dma_start(ra, roinat[b * N : b * N + 128, :])
            rb = rstr.tile([128, RNN], EIN, tag="rb", name="rb")
            nc.scalar.dma_start(
                rb[: N - 128, :], roinat[b * N + 128 : (b + 1) * N, :]
            )
            for dg in range(4):
                pe = pvec.tile([1, 512], F32, tag="pv", name="pe_e")
                nc.tensor.matmul(
                    pe, wT0[:, b : b + 1], ra[:, dg * 512 : (dg + 1) * 512],
                    start=True, stop=False,
                )
                nc.tensor.matmul(
                    pe, wT1[: N - 128, b : b + 1],
                    rb[: N - 128, dg * 512 : (dg + 1) * 512],
                    start=False, stop=True,
                )
                grow = work.tile([1, 512], F32, tag="grow", name="grow")
                nc.vector.tensor_copy(grow, pe)
                nc.gpsimd.dma_start(
                    nat_out[b : b + 1, dg * 512 : (dg + 1) * 512], grow
                )

        # ---- S1a: hg projection (gates the global score pass) ----
        hproj(0)

        # ---- S2: global attention scores -> gw (per-pair softmax) ----
        for pb in range(NPAIR):
            pts = []
            for h in range(2):
                pt = pstr.tile([128, ACH, N], PROI, tag="pt", name="pt")
                nc.scalar.dma_start(pt, proiT2[2 * pb + h])
                pts.append(pt)
            ps_s = pvec.tile([1, GRP], F32, tag="pv", name="ps_s")
            for ch in range(ACH):
                tt = ttp.tile([128, GRP], BF16, tag="tt", bufs=4, name="tt")
                for h in range(2):
                    b = 2 * pb + h
                    nc.scalar.activation(
                        tt[:, h * N : (h + 1) * N], pts[h][:, ch, :], AF.Tanh,
                        bias=hT4[:, 0, ch, b : b + 1], scale=1.0,
                    )
                nc.tensor.matmul(
                    ps_s, wa_sb[:, 0, ch : ch + 1], tt,
                    start=(ch == 0), stop=(ch == ACH - 1),
                )
            for h in range(2):
                b = 2 * pb + h
                softmax_row(
                    ps_s[:, h * N : (h + 1) * N], gw_o[b : b + 1, :],
                    gwT0, gwT1, b,
                )

        # ---- S4: global_out rows (high priority: fills the PE early) ----
        gnat = work.tile([BL, RNN], F32, tag="natrow", name="gnat")
        for b in range(BL):
            einsum_b(b, gwT0, gwT1, gnat)
        for kt in range(KT):
            transpose_cb(gnat[:, kt * 128 : (kt + 1) * 128], globT[:, kt, :])

        # ---- S5: context attention (emitted before S3 => higher priority) ----
        hproj(1)
        r2 = const.tile([128, KT, 2 * BL], SML, tag="r2")
        for kt in range(KT):
            r2v = r2[:, kt, :].rearrange("p (b k) -> p b k", k=2)
            nc.vector.tensor_copy(r2v[:, :, 0], globT[:, kt, :])
            nc.vector.tensor_copy(r2v[:, :, 1], ctxT_sb[:, kt, :])

        def kv2_attention(stat_sb, wj, vbj, hj, score_j, mask_sb, w_out_dram):
            """2-key attention over stat_sb [128, KT, 32] (d x (b,k))."""
            vnat = work.tile([2 * BL, ATT], F32, tag="projnat", name="vnat")
            for ng in range(2):
                ps = pmm.tile([2 * BL, 512], F32, tag="pmm", name="ps_v")
                for kh in range(2):
                    wt = wstr.tile(
                        [128, KT // 2, 512], SML, tag="wst", name="wt_v"
                    )
                    nc.sync.dma_start(wt, wv2t[wj, ng, kh])
                    for k8 in range(KT // 2):
                        kt = kh * 8 + k8
                        nc.tensor.matmul(
                            ps, stat_sb[:, kt, :], wt[:, k8, :],
                            start=(kt == 0), stop=(kt == KT - 1),
                        )
                nc.vector.tensor_copy(vnat[:, ng * 512 : (ng + 1) * 512], ps)
            ps_c = pvec.tile([1, 2 * BL], F32, tag="pv", name="ps_c")
            for ch in range(ACH):
                vT = ttp.tile([128, 2 * BL], F32, tag="vT", name="vT")
                transpose_cb(
                    vnat[:, ch * 128 : (ch + 1) * 128], vT,
                    bias=wvb_sb[:, vbj, ch : ch + 1],
                )
                ta = ttp.tile([128, 2 * BL], F32, tag="ta", name="ta")
                nc.vector.tensor_add(
                    ta.rearrange("p (b k) -> p b k", k=2),
                    vT.rearrange("p (b k) -> p b k", k=2),
                    hT4[:, hj, ch, :].unsqueeze(2).broadcast_to([128, BL, 2]),
                )
                tb = ttp.tile([128, 2 * BL], BF16, tag="tb", name="tb")
                nc.scalar.activation(tb, ta, AF.Tanh)
                nc.tensor.matmul(
                    ps_c, wa_sb[:, score_j, ch : ch + 1], tb,
                    start=(ch == 0), stop=(ch == ACH - 1),
                )
            ew = ttp.tile([1, 2 * BL], F32, tag="ew", name="ew")
            nc.scalar.activation(ew, ps_c, AF.Exp)
            if mask_sb is not None:
                mw = ttp.tile([1, 2 * BL], F32, tag="mw", name="mw")
                nc.vector.tensor_mul(mw, ew, mask_sb)
                ew = mw
            ssum = ttp.tile([1, BL], F32, tag="ssum", name="ssum")
            nc.vector.tensor_reduce(
                out=ssum.unsqueeze(2),
                in_=ew.rearrange("p (b k) -> p b k", k=2),
                op=ALU.add, axis=AXX,
            )
            rcp = ttp.tile([1, BL], F32, tag="rc16", name="rcp2")
            nc.vector.reciprocal(rcp, ssum)
            wgt = ttp.tile([1, 2 * BL], F32, tag="wgt", name="wgt")
            nc.vector.tensor_mul(
                wgt.rearrange("p (b k) -> p b k", k=2),
                ew.rearrange("p (b k) -> p b k", k=2),
                rcp.unsqueeze(2).broadcast_to([1, BL, 2]),
            )
            nc.gpsimd.dma_start(
                w_out_dram.rearrange("b k -> (b k)").unsqueeze(0), wgt
            )
            psr = ptr.tile([128, 128], F32, tag="ptr", name="psr")
            nc.tensor.matmul(
                psr[:, : 2 * BL], ones_c[:, :128], wgt, start=True, stop=True,
            )
            wr = work.tile([128, 2 * BL], F32, tag="wrep", name="wr")
            nc.vector.tensor_copy(wr, psr[:, : 2 * BL])
            return wr

        cwr = kv2_attention(r2, 0, 0, 1, 1, mp_sb, cw_o)
        cwv = cwr.rearrange("p (b k) -> p b k", k=2)
        for kt in range(KT):
            t0 = ttp.tile([128, BL], F32, tag="cmb0", name="t0")
            nc.vector.tensor_mul(t0, globT[:, kt, :], cwv[:, :, 0])
            t1 = ttp.tile([128, BL], F32, tag="cmb1", name="t1")
            nc.vector.tensor_mul(t1, ctxT_sb[:, kt, :], cwv[:, :, 1])
            nc.vector.tensor_add(ctxoT[:, kt, :], t0, t1)

        # ---- S6: hpc = hp + (context_out @ wvp + wvp_b)^T ----
        hproj(2)
        cxb = const.tile([128, KT, BL], BIG, tag="cxb")
        for kt in range(KT):
            nc.vector.tensor_copy(cxb[:, kt, :], ctxoT[:, kt, :])
        cvpn = work.tile([2 * BL, ATT], F32, tag="projnat", name="cvpn")
        for ng in range(2):
            ps = pmm.tile([BL, 512], F32, tag="pmm", name="ps_cv")
            for kt in range(KT):
                nc.tensor.matmul(
                    ps, cxb[:, kt, :], wvp_sb[:, kt, ng * 512 : (ng + 1) * 512],
                    start=(kt == 0), stop=(kt == KT - 1),
                )
            nc.vector.tensor_copy(cvpn[:BL, ng * 512 : (ng + 1) * 512], ps)
        for ch in range(ACH):
            transpose_cb(
                cvpn[:BL, ch * 128 : (ch + 1) * 128], hpc[:, ch, :],
                bias=wvb_sb[:, 1, ch : ch + 1], extra_add=hT4[:, 2, ch, :],
            )

        # ---- S3: big matmul R^T = wvp^T @ roiT -> per-group DRAM (bf16) ----
        for g in range(NGRP):
            rt = rstr.tile([128, KT, GRP], BIG, tag="rt", name="rt")
            nc.sync.dma_start(rt, roiTt[g])
            for m in range(ACH):
                ps = pmm.tile([128, GRP], F32, tag="pmm", name="ps_big")
                for kt in range(KT):
                    nc.tensor.matmul(
                        ps, wvp_sb[:, kt, m * 128 : (m + 1) * 128], rt[:, kt, :],
                        start=(kt == 0), stop=(kt == KT - 1),
                    )
                rc_sb = work.tile([128, GRP], BF16, tag="rtcb", name="rc_sb")
                nc.vector.tensor_copy(rc_sb, ps)
                nc.sync.dma_start(RTg[g][m], rc_sb)

        # ---- S10a: GLU h-half (z_h = h @ glu_w[RNN:, :]) -> DRAM spill ----
        for ng in range(8):
            ps = pmm.tile([BL, 512], F32, tag="pmm", name="ps_zh")
            for kh in range(2):
                gt = wstr.tile([128, KT // 2, 512], GLUDT, tag="wst", name="gt_h")
                nc.sync.dma_start(gt, gluwt[1, ng, kh])
                for k8 in range(KT // 2):
                    kt = kh * 8 + k8
                    nc.tensor.matmul(
                        ps, hT_sb[:, kt, :], gt[:, k8, :],
                        start=(kt == 0), stop=(kt == KT - 1),
                    )
            zt = work.tile([BL, 512], F32, tag="zt", name="zt")
            nc.vector.tensor_copy(zt, ps)
            nc.sync.dma_start(zh_d[:, ng * 512 : (ng + 1) * 512], zt)

        # ---- S7 + S8: composition scores, pw, comp row sums (per pair,
        # pipelined against S3 groups via the per-group RT tiles) ----
        cnat = work.tile([BL, RNN], F32, tag="natrow", name="cnat")
        for pb in range(NPAIR):
            ps_s = pvec.tile([1, GRP], F32, tag="pv", name="ps_s7")
            for ch in range(ACH):
                rtt = pstr.tile([128, GRP], BF16, tag="rtt", name="rtt")
                nc.scalar.dma_start(rtt, RTg[pb][ch])
                tt = ttp.tile([128, GRP], BF16, tag="tt", bufs=4, name="tt7")
                for h in range(2):
                    b = 2 * pb + h
                    nc.scalar.activation(
                        tt[:, h * N : (h + 1) * N], rtt[:, h * N : (h + 1) * N],
                        AF.Tanh, bias=hpc[:, ch, b : b + 1], scale=-1.0,
                    )
                nc.tensor.matmul(
                    ps_s, wa_sb[:, 2, ch : ch + 1], tt,
                    start=(ch == 0), stop=(ch == ACH - 1),
                )
            for h in range(2):
                b = 2 * pb + h
                softmax_row(
                    ps_s[:, h * N : (h + 1) * N], pw_o[b : b + 1, :],
                    pwT0, pwT1, b,
                )
            for h in range(2):
                einsum_b(2 * pb + h, pwT0, pwT1, cnat)

        # ---- S8 tail: comp_out^T = ctxo^T - einsum ----
        for kt in range(KT):
            tr = ptr.tile([128, 128], F32, tag="ptr", name="tr8")
            nc.tensor.transpose(
                tr[:, :BL], cnat[:, kt * 128 : (kt + 1) * 128], ident[:BL, :BL]
            )
            nc.vector.tensor_sub(compT[:, kt, :], ctxoT[:, kt, :], tr[:, :BL])

        # ---- S9: output attention ----
        hproj(3)
        of2 = const.tile([128, KT, 2 * BL], SML, tag="of2")
        for kt in range(KT):
            ofv = of2[:, kt, :].rearrange("p (b k) -> p b k", k=2)
            nc.vector.tensor_copy(ofv[:, :, 0], globT[:, kt, :])
            nc.vector.tensor_copy(ofv[:, :, 1], compT[:, kt, :])
        owr = kv2_attention(of2, 1, 2, 3, 3, None, ow_o)
        owv = owr.rearrange("p (b k) -> p b k", k=2)
        for kt in range(KT):
            t0 = ttp.tile([128, BL], F32, tag="cmb0", name="t0b")
            nc.vector.tensor_mul(t0, globT[:, kt, :], owv[:, :, 0])
            t1 = ttp.tile([128, BL], F32, tag="cmb1", name="t1b")
            nc.vector.tensor_mul(t1, compT[:, kt, :], owv[:, :, 1])
            nc.vector.tensor_add(outT[:, kt, :], t0, t1)

        # output rows (natural layout) to DRAM
        onat = work.tile([BL, RNN], F32, tag="onat")
        for kt in range(KT):
            tr = ptr.tile([128, 128], F32, tag="ptr", name="tr9")
            nc.tensor.transpose(tr[:BL, :], outT[:, kt, :], ident[:128, :128])
            nc.vector.tensor_copy(onat[:, kt * 128 : (kt + 1) * 128], tr[:BL, :])
        nc.sync.dma_start(on_o, onat)

        # ---- S10b: GLU output-half + gate ----
        outTg = const.tile([128, KT, BL], GLUDT, tag="outTg")
        for kt in range(KT):
            nc.vector.tensor_copy(outTg[:, kt, :], outT[:, kt, :])
        for ngx in range(4):
            za = work.tile([BL, 512], F32, tag="za", name="za")
            zb = work.tile([BL, 512], F32, tag="zb", name="zb")
            for half, zdst in ((0, za), (1, zb)):
                ng = ngx + 4 * half
                ps = pmm.tile([BL, 512], F32, tag="pmm", name="ps_z")
                for kh in range(2):
                    gt = wstr.tile(
                        [128, KT // 2, 512], GLUDT, tag="wst", name="gt_o"
                    )
                    nc.sync.dma_start(gt, gluwt[0, ng, kh])
                    for k8 in range(KT // 2):
                        kt = kh * 8 + k8
                        nc.tensor.matmul(
                            ps, outTg[:, kt, :], gt[:, k8, :],
                            start=(kt == 0), stop=False,
                        )
                gb = wstr.tile([1, 512], GLUDT, tag="gbt", name="gb")
                nc.sync.dma_start(gb, glub[:, ng * 512 : (ng + 1) * 512])
                nc.tensor.matmul(ps, ones_g, gb, start=False, stop=True)
                zhp = work.tile([BL, 512], F32, tag="zhp", name="zhp")
                nc.sync.dma_start(zhp, zh_d[:, ng * 512 : (ng + 1) * 512])
                nc.vector.tensor_add(zdst, ps, zhp)
            sg = work.tile([BL, 512], F32, tag="sg", name="sg")
            nc.scalar.activation(sg, zb, AF.Sigmoid)
            xt = work.tile([BL, 512], F32, tag="xt", name="xt")
            nc.vector.tensor_mul(xt, za, sg)
            nc.sync.dma_start(xs_o[:, ngx * 512 : (ngx + 1) * 512], xt)

    nc.compile()
    return nc


_NC_CACHE = None


def _get_nc():
    global _NC_CACHE
    if _NC_CACHE is None:
        _NC_CACHE = build()
    return _NC_CACHE


def _tile_w(w):
    """[2048, 1024] -> [2(ng), 2(kh), 128(p), 8(k8), 512(c)] DMA-tiled."""
    a = w.reshape(2, 8, 128, 2, 512)  # [kh, k8, p, ng, c]
    return a.transpose(3, 0, 2, 1, 4)  # [ng, kh, p, k8, c]


def prep_inputs(
    h, roi_feats, p_roi_feats, mask, context,
    whg_w, whg_b, wag_w, wag_b,
    whc_w, whc_b, wvc_w, wvc_b, wac_w, wac_b,
    whp_w, whp_b, wvp_w, wvp_b, wap_w, wap_b,
    wvo_w, wvo_b, who_w, who_b, wao_w, wao_b,
    glu_w, glu_b,
):
    """Build the 8 per-core input maps (host-side sharding + layout)."""
    h = np.asarray(h, np.float32)
    roi = np.asarray(roi_feats, np.float32)
    proi = np.asarray(p_roi_feats, np.float32)
    mask = np.asarray(mask)
    context = np.asarray(context, np.float32)

    wh4t = _cast(
        np.stack([_tile_w(np.asarray(w)) for w in (whg_w, whc_w, whp_w, who_w)]),
        SML,
    )
    wv2t = _cast(np.stack([_tile_w(np.asarray(w)) for w in (wvc_w, wvo_w)]), SML)
    wa4 = _cast(np.stack([wag_w, wac_w, wap_w, wao_w]), BF16)
    wb4 = _cast(np.stack([whg_b, whc_b, whp_b, who_b]), F32)
    wvb3 = _cast(np.stack([wvc_b, wvp_b, wvo_b]), F32)
    wvp = _cast(wvp_w, BIG)
    # gluwt[half, ng, kh, p, k8, c]; half 0 = output rows (0:2048)
    glw = np.asarray(glu_w, np.float32)
    gluwt = _cast(
        glw.reshape(2, 2, 8, 128, 8, 512).transpose(0, 4, 1, 3, 2, 5), GLUDT
    )
    glub = _cast(np.asarray(glu_b, np.float32).reshape(1, -1), GLUDT)

    # NOTE: wag_b/wac_b/wap_b/wao_b shift scores by a constant, which cancels
    # in the softmax (also under the mask-renormalization), so they are unused.

    in_maps = []
    for c in range(NCORES):
        rows = slice(c * BL, (c + 1) * BL)
        roi_bf = _cast(roi[rows].reshape(ROWS, RNN), EIN)
        # [NGRP, 128(p), KT, GRP]: A[g, p, kt, r] = roi_c[g*GRP + r, kt*128 + p]
        roiTt = np.ascontiguousarray(
            roi_bf.reshape(NGRP, GRP, KT, 128).transpose(0, 3, 2, 1)
        )
        # proiT2[b, p, ch, n] = proi[b, n, ch*128 + p]
        proiT2 = np.ascontiguousarray(
            _cast(proi[rows], PROI).reshape(BL, N, ACH, 128).transpose(0, 3, 2, 1)
        )
        cm = (mask[rows] > 0).astype(np.float32)
        mpv = np.stack([np.ones(BL, np.float32), cm], axis=1).reshape(1, 2 * BL)
        in_maps.append(
            dict(
                roiTt=roiTt,
                roinat=roi_bf,
                proiT2=proiT2,
                hT=_cast(h[rows].T, SML),
                ctxT=_cast(context[rows].T, F32),
                wvp=wvp, wh4t=wh4t, wv2t=wv2t, wa4=wa4, wb4=wb4, wvb3=wvb3,
                gluwt=gluwt, glub=glub, mp=mpv,
            )
        )
    return in_maps


def run_on_device(in_maps, trace=False):
    nc = _get_nc()
    return run_bass_kernel_spmd(nc, in_maps, list(range(NCORES)), trace=trace)


def assemble(results):
    x = np.concatenate([r["xs"] for r in results], axis=0).astype(np.float32)
    output = np.concatenate([r["onat"] for r in results], axis=0).astype(np.float32)
    gw = np.concatenate([r["gw"] for r in results], axis=0).astype(np.float32)
    cw = np.concatenate([r["cw"] for r in results], axis=0).astype(np.float32)
    pw = np.concatenate([r["pw"] for r in results], axis=0).astype(np.float32)
    ow = np.concatenate([r["ow"] for r in results], axis=0).astype(np.float32)
    return (x, output, gw, cw, pw, ow)


def kernel(**inputs):
    in_maps = prep_inputs(**inputs)
    res = run_on_device(in_maps, trace=False)
    return assemble(res.results)


# revision 11
# speedup vs baseline: 1.4943x; 1.2147x over previous
"""Trainium2 Bass kernel for nn_Attention_9457517985916.

Multi-stage attention (Global/Context/Composition/Output) + GLU fusion.
B=128, N=196, RNN=2048, ATT=1024 on 8 NeuronCores, data-parallel over B
(16 rows per core, weights replicated).

Layout strategy: every bulk stream is pre-tiled on the host so each DMA
lands in SBUF with maximal-contiguity descriptors (KB-scale per-partition
runs instead of the 392..784B runs a strided read would produce):

  - roiTt  [8, 128, 16, 392]      per-group d-major roi rows (big matmul)
  - roinat [3136, 2048]           n-major rows (attention-weighted sums)
  - proiT2 [16, 128, 8, 196]      att-major p_roi (global score pass)
  - wh4t/wv2t/gluwt               weight chunks pre-arranged per DMA tile

Compute structure per core:
  R^T = wvp^T @ roiT (bf16, fp32 accumulate) spills per-group to DRAM;
  score passes are ACT tanh(in*scale + bias_col) followed by float32r
  matmuls against the wa vectors; softmaxes are batched to avoid ACT
  table reloads; the attention-weighted row sums run as (196->2)-chunk
  PE matmuls over natural-layout roi; GLU runs in two halves (h-half
  early, output-half at the tail). No collectives; every graded output
  is a per-core row slice the host reassembles.
"""

from contextlib import ExitStack

import numpy as np
import ml_dtypes

import concourse.bass as bass
import concourse.tile as tile
from concourse import bacc, mybir
from concourse.bass_utils import run_bass_kernel_spmd
from concourse.masks import make_identity

F32 = mybir.dt.float32
BF16 = mybir.dt.bfloat16
F32R = mybir.dt.float32r
AF = mybir.ActivationFunctionType
ALU = mybir.AluOpType
AXX = mybir.AxisListType.X

B, N, RNN, ATT = 128, 196, 2048, 1024
NCORES = 8
BL = B // NCORES          # 16 rows per core
ROWS = BL * N             # 3136
KT = RNN // 128           # 16 d-chunks
ACH = ATT // 128          # 8 att-chunks
NPAIR = BL // 2           # 8 row pairs
GRP = 2 * N               # 392 big-matmul rows per group (2 batch rows)
NGRP = ROWS // GRP        # 8

# dtype knobs
BIG = BF16     # roiT / wvp / big matmul + cvp
EIN = BF16     # roinat / einsum matmuls
PROI = BF16    # p_roi stream
SML = BF16     # wh4 / wv2 / hT projections
GLUDT = BF16   # glu weights + glu matmuls

_NP = {F32: np.float32, F32R: np.float32, BF16: ml_dtypes.bfloat16}


def _cast(a, dt):
    return np.ascontiguousarray(np.asarray(a), dtype=_NP[dt])


def build():
    nc = bacc.Bacc("TRN2", target_bir_lowering=False, debug=False)
    dti = lambda n, s, d: nc.dram_tensor(n, s, d, kind="ExternalInput").ap()
    dto = lambda n, s, d: nc.dram_tensor(n, s, d, kind="ExternalOutput").ap()

    roiTt = dti("roiTt", [NGRP, 128, KT, GRP], BIG)
    roinat = dti("roinat", [ROWS, RNN], EIN)
    proiT2 = dti("proiT2", [BL, 128, ACH, N], PROI)
    hT = dti("hT", [RNN, BL], SML)
    ctxT = dti("ctxT", [RNN, BL], F32)
    wvp = dti("wvp", [RNN, ATT], BIG)
    wh4t = dti("wh4t", [4, 2, 2, 128, KT // 2, 512], SML)
    wv2t = dti("wv2t", [2, 2, 2, 128, KT // 2, 512], SML)
    gluwt = dti("gluwt", [2, 8, 2, 128, KT // 2, 512], GLUDT)
    wa4 = dti("wa4", [4, ATT], BF16)
    wb4 = dti("wb4", [4, ATT], F32)
    wvb3 = dti("wvb3", [3, ATT], F32)
    glub = dti("glub", [1, 2 * RNN], GLUDT)
    mp = dti("mp", [1, 2 * BL], F32)

    xs_o = dto("xs", [BL, RNN], F32)       # [16, 2048] x rows
    on_o = dto("onat", [BL, RNN], F32)     # [16, 2048] output rows
    gw_o = dto("gw", [BL, N], F32)
    cw_o = dto("cw", [BL, 2], F32)
    pw_o = dto("pw", [BL, N], F32)
    ow_o = dto("ow", [BL, 2], F32)

    with tile.TileContext(nc) as tc, ExitStack() as ctx:
        const = ctx.enter_context(tc.tile_pool(name="const", bufs=1))
        wstr = ctx.enter_context(tc.tile_pool(name="wstr", bufs=2))
        rstr = ctx.enter_context(tc.tile_pool(name="rstr", bufs=2))
        pstr = ctx.enter_context(tc.tile_pool(name="pstr", bufs=3))
        work = ctx.enter_context(tc.tile_pool(name="work", bufs=2))
        ttp = ctx.enter_context(tc.tile_pool(name="ttp", bufs=3))
        pmm = ctx.enter_context(tc.tile_pool(name="pmm", bufs=3, space="PSUM"))
        pvec = ctx.enter_context(tc.tile_pool(name="pvec", bufs=3, space="PSUM"))
        ptr = ctx.enter_context(tc.tile_pool(name="ptr", bufs=2, space="PSUM"))
        dram = ctx.enter_context(tc.tile_pool(name="dram", bufs=1, space="DRAM"))

        # ---- constants ----
        ident = const.tile([128, 128], F32, tag="ident")
        make_identity(nc, ident)
        ones_g = const.tile([1, BL], GLUDT, tag="ones_g")
        nc.vector.memset(ones_g, 1.0)
        ones_c = const.tile([1, 128], F32, tag="ones_c")
        nc.vector.memset(ones_c, 1.0)

        wa_sb = const.tile([128, 4, ACH], BF16, tag="wa_sb")
        nc.sync.dma_start(wa_sb, wa4.rearrange("j (ch p) -> p j ch", p=128))
        wb_sb = const.tile([128, 4, ACH], F32, tag="wb_sb")
        nc.sync.dma_start(wb_sb, wb4.rearrange("j (ch p) -> p j ch", p=128))
        wvb_sb = const.tile([128, 3, ACH], F32, tag="wvb_sb")
        nc.sync.dma_start(wvb_sb, wvb3.rearrange("j (ch p) -> p j ch", p=128))
        mp_sb = const.tile([1, 2 * BL], F32, tag="mp_sb")
        nc.sync.dma_start(mp_sb, mp)

        hT_sb = const.tile([128, KT, BL], SML, tag="hT_sb")
        nc.sync.dma_start(hT_sb, hT.rearrange("(kt p) b -> p kt b", p=128))
        ctxT_sb = const.tile([128, KT, BL], F32, tag="ctxT_sb")
        nc.sync.dma_start(ctxT_sb, ctxT.rearrange("(kt p) b -> p kt b", p=128))
        wvp_sb = const.tile([128, KT, ATT], BIG, tag="wvp_sb")
        nc.sync.dma_start(wvp_sb, wvp.rearrange("(kt p) m -> p kt m", p=128))

        # persistent state
        hT4 = const.tile([128, 4, ACH, BL], F32, tag="hT4")
        globT = const.tile([128, KT, BL], F32, tag="globT")
        ctxoT = const.tile([128, KT, BL], F32, tag="ctxoT")
        hpc = const.tile([128, ACH, BL], F32, tag="hpc")
        compT = const.tile([128, KT, BL], F32, tag="compT")
        outT = const.tile([128, KT, BL], F32, tag="outT")

        RTg = [
            dram.tile([ACH, 128, GRP], BF16, tag=f"RTg{g}", name=f"RTg{g}")
            for g in range(NGRP)
        ]
        zh_d = dram.tile([BL, 2 * RNN], F32, tag="zh_d")

        gwT0 = const.tile([128, BL], EIN, tag="gwT0")
        gwT1 = const.tile([128, BL], EIN, tag="gwT1")
        pwT0 = const.tile([128, BL], EIN, tag="pwT0")
        pwT1 = const.tile([128, BL], EIN, tag="pwT1")

        # fused psum->sbuf transpose helper (DVE copyback; keeps ACT free)
        def transpose_cb(in_sb, out_ap, bias=None, extra_add=None):
            p_in = in_sb.shape[0]
            f_in = in_sb.shape[1]
            tr = ptr.tile([128, 128], F32, tag="ptr")
            trv = tr[:f_in, :p_in]
            nc.tensor.transpose(trv, in_sb, ident[:p_in, :p_in])
            if extra_add is not None:
                if bias is not None:
                    tmp = ttp.tile([128, BL], F32, tag="trtmp")
                    tv = tmp[:f_in, :p_in]
                    nc.vector.tensor_scalar_add(tv, trv, bias[:f_in, :])
                    nc.vector.tensor_add(out_ap, tv, extra_add)
                else:
                    nc.vector.tensor_add(out_ap, trv, extra_add)
            elif bias is not None:
                nc.vector.tensor_scalar_add(out_ap, trv, bias[:f_in, :])
            else:
                nc.vector.tensor_copy(out_ap, trv)

        # ---- h projection j -> hT4[:, j, ch, :] ----
        def hproj(j):
            hn = work.tile([2 * BL, ATT], F32, tag="projnat", name="hn")
            for ng in range(2):
                ps = pmm.tile([BL, 512], F32, tag="pmm", name="ps_h")
                for kh in range(2):
                    wt = wstr.tile(
                        [128, KT // 2, 512], SML, tag="wst", name="wt_h"
                    )
                    nc.sync.dma_start(wt, wh4t[j, ng, kh])
                    for k8 in range(KT // 2):
                        kt = kh * 8 + k8
                        nc.tensor.matmul(
                            ps, hT_sb[:, kt, :], wt[:, k8, :],
                            start=(kt == 0), stop=(kt == KT - 1),
                        )
                nc.vector.tensor_copy(hn[:BL, ng * 512 : (ng + 1) * 512], ps)
            for ch in range(ACH):
                transpose_cb(
                    hn[:BL, ch * 128 : (ch + 1) * 128],
                    hT4[:, j, ch, :],
                    bias=wb_sb[:, j, ch : ch + 1],
                )

        # ---- S3 big-matmul chunk generator: one (g, m) 16-matmul chunk ----
        s3_state = {"g": 0, "m": 0, "rt": None}

        def s3_chunks(n):
            for _ in range(n):
                g, m = s3_state["g"], s3_state["m"]
                if g >= NGRP:
                    return
                if m == 0:
                    rt = rstr.tile([128, KT, GRP], BIG, tag="rt", name="rt")
                    nc.sync.dma_start(rt, roiTt[g])
                    s3_state["rt"] = rt
                rt = s3_state["rt"]
                ps = pmm.tile([128, GRP], F32, tag="pmm", name="ps_big")
                for kt in range(KT):
                    nc.tensor.matmul(
                        ps, wvp_sb[:, kt, m * 128 : (m + 1) * 128], rt[:, kt, :],
                        start=(kt == 0), stop=(kt == KT - 1),
                    )
                rc_sb = work.tile([128, GRP], BF16, tag="rtcb", name="rc_sb")
                nc.vector.tensor_copy(rc_sb, ps)
                nc.sync.dma_start(RTg[g][m], rc_sb)
                if m + 1 == ACH:
                    s3_state["g"], s3_state["m"] = g + 1, 0
                else:
                    s3_state["m"] = m + 1

        # ---- S10a GLU h-half block (pure filler work) ----
        def s10a_block(ng):
            ps = pmm.tile([BL, 512], F32, tag="pmm", name="ps_zh")
            for kh in range(2):
                gt = wstr.tile([128, KT // 2, 512], GLUDT, tag="wst", name="gt_h")
                nc.sync.dma_start(gt, gluwt[1, ng, kh])
                for k8 in range(KT // 2):
                    kt = kh * 8 + k8
                    nc.tensor.matmul(
                        ps, hT_sb[:, kt, :], gt[:, k8, :],
                        start=(kt == 0), stop=(kt == KT - 1),
                    )
            zt = work.tile([BL, 512], F32, tag="zt", name="zt")
            nc.vector.tensor_copy(zt, ps)
            nc.sync.dma_start(zh_d[:, ng * 512 : (ng + 1) * 512], zt)

        # softmax of one [1, N] psum slice -> weight row + transposed columns
        def softmax_row(sl, w_row, wT0, wT1, b):
            nm = ttp.tile([1, 1], F32, tag="nm", name="nm")
            nc.vector.tensor_reduce(
                out=nm, in_=sl, op=ALU.max, axis=AXX, negate=True
            )
            ex = ttp.tile([1, N], F32, tag="ex", bufs=2, name="ex")
            se = ttp.tile([1, 1], F32, tag="se", name="se")
            nc.scalar.activation(ex, sl, AF.Exp, bias=nm, scale=1.0, accum_out=se)
            rcp = ttp.tile([1, 1], F32, tag="rc", name="rcp")
            nc.vector.reciprocal(rcp, se)
            wf = ttp.tile([1, N], F32, tag="wf", bufs=2, name="wf")
            nc.vector.tensor_scalar_mul(wf, ex, rcp)
            nc.gpsimd.dma_start(w_row, wf)
            tra = ptr.tile([128, 128], F32, tag="ptr", name="tra")
            nc.tensor.transpose(tra[:128, :1], wf[:, :128], ident[:1, :1])
            nc.vector.tensor_copy(wT0[:, b : b + 1], tra[:128, :1])
            trb = ptr.tile([128, 128], F32, tag="ptr", name="trb")
            nc.tensor.transpose(trb[: N - 128, :1], wf[:, 128:N], ident[:1, :1])
            nc.vector.tensor_copy(wT1[: N - 128, b : b + 1], trb[: N - 128, :1])

        # einsum: nat_out[b, :] = sum_n w[b, n] * roi[b, n, :]
        def einsum_b(b, wT0, wT1, nat_out):
            ra = rstr.tile([128, RNN], EIN, tag="ra", name="ra")
            nc.scalar.dma_start(ra, roinat[b * N : b * N + 128, :])
            rb = rstr.tile([128, RNN], EIN, tag="rb", name="rb")
            nc.scalar.dma_start(
                rb[: N - 128, :], roinat[b * N + 128 : (b + 1) * N, :]
            )
            for dg in range(4):
                pe = pvec.tile([1, 512], F32, tag="pv", name="pe_e")
                nc.tensor.matmul(
                    pe, wT0[:, b : b + 1], ra[:, dg * 512 : (dg + 1) * 512],
                    start=True, stop=False,
                )
                nc.tensor.matmul(
                    pe, wT1[: N - 128, b : b + 1],
                    rb[: N - 128, dg * 512 : (dg + 1) * 512],
                    start=False, stop=True,
                )
                grow = work.tile([1, 512], F32, tag="grow", name="grow")
                nc.vector.tensor_copy(grow, pe)
                nc.gpsimd.dma_start(
                    nat_out[b : b + 1, dg * 512 : (dg + 1) * 512], grow
                )

        # ---- S1a: hg projection (gates the global score pass) ----
        hproj(0)

        # ---- S2: global attention scores -> gw; S3 chunks as filler ----
        for pb in range(NPAIR):
            pts = []
            for h in range(2):
                pt = pstr.tile([128, ACH, N], PROI, tag="pt", name="pt")
                nc.scalar.dma_start(pt, proiT2[2 * pb + h])
                pts.append(pt)
            tts = []
            for ch in range(ACH):
                tt = ttp.tile([128, GRP], BF16, tag="tt", bufs=8, name="tt")
                for h in range(2):
                    b = 2 * pb + h
                    nc.scalar.activation(
                        tt[:, h * N : (h + 1) * N], pts[h][:, ch, :], AF.Tanh,
                        bias=hT4[:, 0, ch, b : b + 1], scale=1.0,
                    )
                tts.append(tt)
            s3_chunks(1)
            ps_s = pvec.tile([1, GRP], F32, tag="pv", name="ps_s")
            for ch in range(ACH):
                nc.tensor.matmul(
                    ps_s, wa_sb[:, 0, ch : ch + 1], tts[ch],
                    start=(ch == 0), stop=(ch == ACH - 1),
                )
            for h in range(2):
                b = 2 * pb + h
                softmax_row(
                    ps_s[:, h * N : (h + 1) * N], gw_o[b : b + 1, :],
                    gwT0, gwT1, b,
                )
            s3_chunks(1)
            if pb % 2 == 1:
                s10a_block(pb // 2)

        # ---- S4: global_out row sums, S3 filler between rows ----
        gnat = work.tile([BL, RNN], F32, tag="natrow", name="gnat")
        for b in range(BL):
            einsum_b(b, gwT0, gwT1, gnat)
            s3_chunks(1)
        for kt in range(KT):
            transpose_cb(gnat[:, kt * 128 : (kt + 1) * 128], globT[:, kt, :])

        # ---- S5: context attention ----
        hproj(1)
        s3_chunks(2)
        r2 = const.tile([128, KT, 2 * BL], SML, tag="r2")
        for kt in range(KT):
            r2v = r2[:, kt, :].rearrange("p (b k) -> p b k", k=2)
            nc.vector.tensor_copy(r2v[:, :, 0], globT[:, kt, :])
            nc.vector.tensor_copy(r2v[:, :, 1], ctxT_sb[:, kt, :])

        def kv2_attention(stat_sb, wj, vbj, hj, score_j, mask_sb, w_out_dram):
            """2-key attention over stat_sb [128, KT, 32] (d x (b,k))."""
            vnat = work.tile([2 * BL, ATT], F32, tag="projnat", name="vnat")
            for ng in range(2):
                ps = pmm.tile([2 * BL, 512], F32, tag="pmm", name="ps_v")
                for kh in range(2):
                    wt = wstr.tile(
                        [128, KT // 2, 512], SML, tag="wst", name="wt_v"
                    )
                    nc.sync.dma_start(wt, wv2t[wj, ng, kh])
                    for k8 in range(KT // 2):
                        kt = kh * 8 + k8
                        nc.tensor.matmul(
                            ps, stat_sb[:, kt, :], wt[:, k8, :],
                            start=(kt == 0), stop=(kt == KT - 1),
                        )
                nc.vector.tensor_copy(vnat[:, ng * 512 : (ng + 1) * 512], ps)
            tbs = []
            for ch in range(ACH):
                vT = ttp.tile([128, 2 * BL], F32, tag="vT", bufs=8, name="vT")
                transpose_cb(
                    vnat[:, ch * 128 : (ch + 1) * 128], vT,
                    bias=wvb_sb[:, vbj, ch : ch + 1],
                )
                ta = ttp.tile([128, 2 * BL], F32, tag="ta", bufs=8, name="ta")
                nc.vector.tensor_add(
                    ta.rearrange("p (b k) -> p b k", k=2),
                    vT.rearrange("p (b k) -> p b k", k=2),
                    hT4[:, hj, ch, :].unsqueeze(2).broadcast_to([128, BL, 2]),
                )
                tb = ttp.tile([128, 2 * BL], BF16, tag="tb", bufs=8, name="tb")
                nc.scalar.activation(tb, ta, AF.Tanh)
                tbs.append(tb)
            s3_chunks(1)
            ps_c = pvec.tile([1, 2 * BL], F32, tag="pv", name="ps_c")
            for ch in range(ACH):
                nc.tensor.matmul(
                    ps_c, wa_sb[:, score_j, ch : ch + 1], tbs[ch],
                    start=(ch == 0), stop=(ch == ACH - 1),
                )
            ew = ttp.tile([1, 2 * BL], F32, tag="ew", name="ew")
            nc.scalar.activation(ew, ps_c, AF.Exp)
            if mask_sb is not None:
                mw = ttp.tile([1, 2 * BL], F32, tag="mw", name="mw")
                nc.vector.tensor_mul(mw, ew, mask_sb)
                ew = mw
            ssum = ttp.tile([1, BL], F32, tag="ssum", name="ssum")
            nc.vector.tensor_reduce(
                out=ssum.unsqueeze(2),
                in_=ew.rearrange("p (b k) -> p b k", k=2),
                op=ALU.add, axis=AXX,
            )
            rcp = ttp.tile([1, BL], F32, tag="rc16", name="rcp2")
            nc.vector.reciprocal(rcp, ssum)
            wgt = ttp.tile([1, 2 * BL], F32, tag="wgt", name="wgt")
            nc.vector.tensor_mul(
                wgt.rearrange("p (b k) -> p b k", k=2),
                ew.rearrange("p (b k) -> p b k", k=2),
                rcp.unsqueeze(2).broadcast_to([1, BL, 2]),
            )
            nc.gpsimd.dma_start(
                w_out_dram.rearrange("b k -> (b k)").unsqueeze(0), wgt
            )
            psr = ptr.tile([128, 128], F32, tag="ptr", name="psr")
            nc.tensor.matmul(
                psr[:, : 2 * BL], ones_c[:, :128], wgt, start=True, stop=True,
            )
            wr = work.tile([128, 2 * BL], F32, tag="wrep", name="wr")
            nc.vector.tensor_copy(wr, psr[:, : 2 * BL])
            return wr

        cwr = kv2_attention(r2, 0, 0, 1, 1, mp_sb, cw_o)
        cwv = cwr.rearrange("p (b k) -> p b k", k=2)
        for kt in range(KT):
            t0 = ttp.tile([128, BL], F32, tag="cmb0", name="t0")
            nc.vector.tensor_mul(t0, globT[:, kt, :], cwv[:, :, 0])
            t1 = ttp.tile([128, BL], F32, tag="cmb1", name="t1")
            nc.vector.tensor_mul(t1, ctxT_sb[:, kt, :], cwv[:, :, 1])
            nc.vector.tensor_add(ctxoT[:, kt, :], t0, t1)
        s3_chunks(2)

        # ---- S6: hpc = hp + (context_out @ wvp + wvp_b)^T ----
        hproj(2)
        cxb = const.tile([128, KT, BL], BIG, tag="cxb")
        for kt in range(KT):
            nc.vector.tensor_copy(cxb[:, kt, :], ctxoT[:, kt, :])
        s3_chunks(2)
        cvpn = work.tile([2 * BL, ATT], F32, tag="projnat", name="cvpn")
        for ng in range(2):
            ps = pmm.tile([BL, 512], F32, tag="pmm", name="ps_cv")
            for kt in range(KT):
                nc.tensor.matmul(
                    ps, cxb[:, kt, :], wvp_sb[:, kt, ng * 512 : (ng + 1) * 512],
                    start=(kt == 0), stop=(kt == KT - 1),
                )
            nc.vector.tensor_copy(cvpn[:BL, ng * 512 : (ng + 1) * 512], ps)
        for ch in range(ACH):
            transpose_cb(
                cvpn[:BL, ch * 128 : (ch + 1) * 128], hpc[:, ch, :],
                bias=wvb_sb[:, 1, ch : ch + 1], extra_add=hT4[:, 2, ch, :],
            )
        s3_chunks(2)

        # ---- S7 + S8: composition scores, pw, comp row sums (per pair,
        # interleaved with the remaining S3 chunks) ----
        cnat = work.tile([BL, RNN], F32, tag="natrow", name="cnat")
        for pb in range(NPAIR):
            tts = []
            for ch in range(ACH):
                rtt = pstr.tile([128, GRP], BF16, tag="rtt", name="rtt")
                nc.scalar.dma_start(rtt, RTg[pb][ch])
                tt = ttp.tile([128, GRP], BF16, tag="tt", bufs=8, name="tt7")
                for h in range(2):
                    b = 2 * pb + h
                    nc.scalar.activation(
                        tt[:, h * N : (h + 1) * N], rtt[:, h * N : (h + 1) * N],
                        AF.Tanh, bias=hpc[:, ch, b : b + 1], scale=-1.0,
                    )
                tts.append(tt)
            s3_chunks(2)
            ps_s = pvec.tile([1, GRP], F32, tag="pv", name="ps_s7")
            for ch in range(ACH):
                nc.tensor.matmul(
                    ps_s, wa_sb[:, 2, ch : ch + 1], tts[ch],
                    start=(ch == 0), stop=(ch == ACH - 1),
                )
            for h in range(2):
                b = 2 * pb + h
                softmax_row(
                    ps_s[:, h * N : (h + 1) * N], pw_o[b : b + 1, :],
                    pwT0, pwT1, b,
                )
            for h in range(2):
                einsum_b(2 * pb + h, pwT0, pwT1, cnat)
                s3_chunks(1)
            if pb % 2 == 1:
                s10a_block(4 + pb // 2)

        s3_chunks(99)

        # ---- S8 tail: comp_out^T = ctxo^T - einsum ----
        for kt in range(KT):
            tr = ptr.tile([128, 128], F32, tag="ptr", name="tr8")
            nc.tensor.transpose(
                tr[:, :BL], cnat[:, kt * 128 : (kt + 1) * 128], ident[:BL, :BL]
            )
            nc.vector.tensor_sub(compT[:, kt, :], ctxoT[:, kt, :], tr[:, :BL])

        # ---- S9: output attention ----
        hproj(3)
        of2 = const.tile([128, KT, 2 * BL], SML, tag="of2")
        for kt in range(KT):
            ofv = of2[:, kt, :].rearrange("p (b k) -> p b k", k=2)
            nc.vector.tensor_copy(ofv[:, :, 0], globT[:, kt, :])
            nc.vector.tensor_copy(ofv[:, :, 1], compT[:, kt, :])
        owr = kv2_attention(of2, 1, 2, 3, 3, None, ow_o)
        owv = owr.rearrange("p (b k) -> p b k", k=2)
        for kt in range(KT):
            t0 = ttp.tile([128, BL], F32, tag="cmb0", name="t0b")
            nc.vector.tensor_mul(t0, globT[:, kt, :], owv[:, :, 0])
            t1 = ttp.tile([128, BL], F32, tag="cmb1", name="t1b")
            nc.vector.tensor_mul(t1, compT[:, kt, :], owv[:, :, 1])
            nc.vector.tensor_add(outT[:, kt, :], t0, t1)

        # output rows (natural layout) to DRAM
        onat = work.tile([BL, RNN], F32, tag="onat")
        for kt in range(KT):
            tr = ptr.tile([128, 128], F32, tag="ptr", name="tr9")
            nc.tensor.transpose(tr[:BL, :], outT[:, kt, :], ident[:128, :128])
            nc.vector.tensor_copy(onat[:, kt * 128 : (kt + 1) * 128], tr[:BL, :])
        nc.sync.dma_start(on_o, onat)

        # ---- S10b: GLU output-half + gate ----
        outTg = const.tile([128, KT, BL], GLUDT, tag="outTg")
        for kt in range(KT):
            nc.vector.tensor_copy(outTg[:, kt, :], outT[:, kt, :])
        for ngx in range(4):
            za = work.tile([BL, 512], F32, tag="za", name="za")
            zb = work.tile([BL, 512], F32, tag="zb", name="zb")
            for half, zdst in ((0, za), (1, zb)):
                ng = ngx + 4 * half
                ps = pmm.tile([BL, 512], F32, tag="pmm", name="ps_z")
                for kh in range(2):
                    gt = wstr.tile(
                        [128, KT // 2, 512], GLUDT, tag="wst", name="gt_o"
                    )
                    nc.sync.dma_start(gt, gluwt[0, ng, kh])
                    for k8 in range(KT // 2):
                        kt = kh * 8 + k8
                        nc.tensor.matmul(
                            ps, outTg[:, kt, :], gt[:, k8, :],
                            start=(kt == 0), stop=False,
                        )
                gb = wstr.tile([1, 512], GLUDT, tag="gbt", name="gb")
                nc.sync.dma_start(gb, glub[:, ng * 512 : (ng + 1) * 512])
                nc.tensor.matmul(ps, ones_g, gb, start=False, stop=True)
                zhp = work.tile([BL, 512], F32, tag="zhp", name="zhp")
                nc.sync.dma_start(zhp, zh_d[:, ng * 512 : (ng + 1) * 512])
                nc.vector.tensor_add(zdst, ps, zhp)
            sg = work.tile([BL, 512], F32, tag="sg", name="sg")
            nc.scalar.activation(sg, zb, AF.Sigmoid)
            xt = work.tile([BL, 512], F32, tag="xt", name="xt")
            nc.vector.tensor_mul(xt, za, sg)
            nc.sync.dma_start(xs_o[:, ngx * 512 : (ngx + 1) * 512], xt)

    nc.compile()
    return nc


_NC_CACHE = None


def _get_nc():
    global _NC_CACHE
    if _NC_CACHE is None:
        _NC_CACHE = build()
    return _NC_CACHE


def _tile_w(w):
    """[2048, 1024] -> [2(ng), 2(kh), 128(p), 8(k8), 512(c)] DMA-tiled."""
    a = w.reshape(2, 8, 128, 2, 512)  # [kh, k8, p, ng, c]
    return a.transpose(3, 0, 2, 1, 4)  # [ng, kh, p, k8, c]


def prep_inputs(
    h, roi_feats, p_roi_feats, mask, context,
    whg_w, whg_b, wag_w, wag_b,
    whc_w, whc_b, wvc_w, wvc_b, wac_w, wac_b,
    whp_w, whp_b, wvp_w, wvp_b, wap_w, wap_b,
    wvo_w, wvo_b, who_w, who_b, wao_w, wao_b,
    glu_w, glu_b,
):
    """Build the 8 per-core input maps (host-side sharding + layout)."""
    h = np.asarray(h, np.float32)
    roi = np.asarray(roi_feats, np.float32)
    proi = np.asarray(p_roi_feats, np.float32)
    mask = np.asarray(mask)
    context = np.asarray(context, np.float32)

    wh4t = _cast(
        np.stack([_tile_w(np.asarray(w)) for w in (whg_w, whc_w, whp_w, who_w)]),
        SML,
    )
    wv2t = _cast(np.stack([_tile_w(np.asarray(w)) for w in (wvc_w, wvo_w)]), SML)
    wa4 = _cast(np.stack([wag_w, wac_w, wap_w, wao_w]), BF16)
    wb4 = _cast(np.stack([whg_b, whc_b, whp_b, who_b]), F32)
    wvb3 = _cast(np.stack([wvc_b, wvp_b, wvo_b]), F32)
    wvp = _cast(wvp_w, BIG)
    # gluwt[half, ng, kh, p, k8, c]; half 0 = output rows (0:2048)
    glw = np.asarray(glu_w, np.float32)
    gluwt = _cast(
        glw.reshape(2, 2, 8, 128, 8, 512).transpose(0, 4, 1, 3, 2, 5), GLUDT
    )
    glub = _cast(np.asarray(glu_b, np.float32).reshape(1, -1), GLUDT)

    # NOTE: wag_b/wac_b/wap_b/wao_b shift scores by a constant, which cancels
    # in the softmax (also under the mask-renormalization), so they are unused.

    in_maps = []
    for c in range(NCORES):
        rows = slice(c * BL, (c + 1) * BL)
        roi_bf = _cast(roi[rows].reshape(ROWS, RNN), EIN)
        # [NGRP, 128(p), KT, GRP]: A[g, p, kt, r] = roi_c[g*GRP + r, kt*128 + p]
        roiTt = np.ascontiguousarray(
            roi_bf.reshape(NGRP, GRP, KT, 128).transpose(0, 3, 2, 1)
        )
        # proiT2[b, p, ch, n] = proi[b, n, ch*128 + p]
        proiT2 = np.ascontiguousarray(
            _cast(proi[rows], PROI).reshape(BL, N, ACH, 128).transpose(0, 3, 2, 1)
        )
        cm = (mask[rows] > 0).astype(np.float32)
        mpv = np.stack([np.ones(BL, np.float32), cm], axis=1).reshape(1, 2 * BL)
        in_maps.append(
            dict(
                roiTt=roiTt,
                roinat=roi_bf,
                proiT2=proiT2,
                hT=_cast(h[rows].T, SML),
                ctxT=_cast(context[rows].T, F32),
                wvp=wvp, wh4t=wh4t, wv2t=wv2t, wa4=wa4, wb4=wb4, wvb3=wvb3,
                gluwt=gluwt, glub=glub, mp=mpv,
            )
        )
    return in_maps


def run_on_device(in_maps, trace=False):
    nc = _get_nc()
    return run_bass_kernel_spmd(nc, in_maps, list(range(NCORES)), trace=trace)


def assemble(results):
    x = np.concatenate([r["xs"] for r in results], axis=0).astype(np.float32)
    output = np.concatenate([r["onat"] for r in results], axis=0).astype(np.float32)
    gw = np.concatenate([r["gw"] for r in results], axis=0).astype(np.float32)
    cw = np.concatenate([r["cw"] for r in results], axis=0).astype(np.float32)
    pw = np.concatenate([r["pw"] for r in results], axis=0).astype(np.float32)
    ow = np.concatenate([r["ow"] for r in results], axis=0).astype(np.float32)
    return (x, output, gw, cw, pw, ow)


def kernel(**inputs):
    in_maps = prep_inputs(**inputs)
    res = run_on_device(in_maps, trace=False)
    return assemble(res.results)
